# revision 2
# baseline (speedup 1.0000x reference)
"""AttnBlock (GroupNorm -> single-head attention over 64x64 pixels -> out conv
-> residual) on 8 Trainium2 NeuronCores.

Sharding: data parallel over batch (B=4) x 2-way split of the query-pixel axis
=> 8 cores, no collectives.  Each core receives its batch element's pixels as
two inputs: xq (its 2048 query columns) and xo (the other 2048 columns).  K/V
are computed over all 4096 pixels in the core-local order [xq | xo] (attention
sums over keys, so key ordering is irrelevant to the output).

All shapes hardcoded: B=4, C=512, H=W=64, N=4096, 32 groups.
"""

import numpy as np

B, C, H, W = 4, 512, 64, 64
N = H * W              # 4096 pixels
NQ = N // 2            # 2048 query pixels per core
NUM_GROUPS = 32
GSIZE = C // NUM_GROUPS  # 16 channels per group
EPS = 1e-6
SCALE = float(C) ** 0.5  # reference multiplies scores by sqrt(C)

P = 128                # partitions
CC = C // P            # 4 channel chunks
QCH = NQ // P          # 16 query chunks per core
NKQ = 1024             # k-columns per score quarter
NQW = N // NKQ         # 4 quarters per query chunk

# "fp32r" = fast reduced-precision fp32 matmul for the q/k/scores path;
# "fp32" = full precision (4x slower on PE for the scores matmuls).
QK_MODE = "fp32r"

_CACHE = {}


def _build(qk_mode, repeats=1):
    from contextlib import ExitStack

    import concourse.bacc as bacc
    import concourse.tile as tile
    from concourse import mybir
    from concourse.masks import make_identity

    dt = mybir.dt
    qk_dt = dt.float32r if qk_mode == "fp32r" else dt.float32

    nc = bacc.Bacc()
    xq_ext = nc.declare_dram_parameter("xq", [C, NQ], dt.float32, isOutput=False)
    xo_ext = nc.declare_dram_parameter("xo", [C, NQ], dt.float32, isOutput=False)
    wqT_ext = nc.declare_dram_parameter("wqT", [C, C], dt.float32, isOutput=False)
    wkT_ext = nc.declare_dram_parameter("wkT", [C, C], dt.float32, isOutput=False)
    wvT_ext = nc.declare_dram_parameter("wvT", [C, C], dt.float32, isOutput=False)
    woT_ext = nc.declare_dram_parameter("woT", [C, C], dt.float32, isOutput=False)
    biases_ext = nc.declare_dram_parameter("biases", [C, 4], dt.float32, isOutput=False)
    gn_ab_ext = nc.declare_dram_parameter("gn_ab", [C, 2], dt.float32, isOutput=False)
    gsel_ext = nc.declare_dram_parameter("gsel", [C, NUM_GROUPS], dt.float32, isOutput=False)
    esel_ext = nc.declare_dram_parameter("esel", [NUM_GROUPS, C], dt.float32, isOutput=False)
    out_ext = nc.declare_dram_parameter("out", [C, NQ], dt.float32, isOutput=True)

    with tile.TileContext(nc) as tc:
        # LEFT side: long-lived pools (whole kernel / attention phase).
        # RIGHT side: phase-scoped pools (GN scratch, conv weights, h).
        for _rep in range(repeats):
            top = ExitStack()
            const = top.enter_context(tc.tile_pool(name="const", bufs=1, side="left"))
            biases_sb = const.tile([P, CC, 4], dt.float32)  # [:, :, 0..3] = bq, bk, bv, bo
            nc.sync.dma_start(out=biases_sb[:], in_=biases_ext.rearrange("(c p) k -> p c k", p=P))
            k_pool = top.enter_context(tc.tile_pool(name="k_pool", bufs=1, side="left"))
            vT_pool = top.enter_context(tc.tile_pool(name="vT_pool", bufs=1, side="left"))

            # ---------------- Phase 1+2: GroupNorm folded into convs ----------------
            # GroupNorm h = a*x + b is folded into the conv weights:
            #   W' = W @ diag(a),  bias' = W @ b + bias
            # so K/V/Q are computed directly from x and h never materializes.
            # Group stats are per channel-chunk (groups never span chunks), so
            # chunk cc's conv matmuls start as soon as its own stats are done.
            bx_sb = const.tile([P, CC, 3], dt.float32)  # folded conv biases q,k,v
            q_pool = top.enter_context(tc.tile_pool(name="q_pool", bufs=1, side="left"))

            hq_stack = ExitStack()
            hq_pool = hq_stack.enter_context(tc.tile_pool(name="hq_pool", bufs=1, side="right"))
            ho_stack = ExitStack()
            ho_pool = ho_stack.enter_context(tc.tile_pool(name="ho_pool", bufs=1, side="right"))
            af_stack = ExitStack()
            affine = af_stack.enter_context(tc.tile_pool(name="affine", bufs=1, side="right"))
            gn_stack = ExitStack()
            stat_pool = gn_stack.enter_context(tc.tile_pool(name="stat_pool", bufs=1, side="right"))
            small = gn_stack.enter_context(tc.tile_pool(name="small", bufs=1, side="right"))
            ps_small = gn_stack.enter_context(
                tc.tile_pool(name="ps_small", bufs=1, space="PSUM", side="right"))

            gsel_sb = small.tile([P, CC, NUM_GROUPS], dt.float32)
            nc.sync.dma_start(out=gsel_sb[:], in_=gsel_ext.rearrange("(c p) g -> p c g", p=P))
            esel_sb = small.tile([NUM_GROUPS, C], dt.float32)
            nc.sync.dma_start(out=esel_sb[:], in_=esel_ext[:])
            gn_ab_sb = small.tile([P, CC, 2], dt.float32)
            nc.sync.dma_start(out=gn_ab_sb[:], in_=gn_ab_ext.rearrange("(c p) k -> p c k", p=P))
            eps_sb = small.tile([NUM_GROUPS, 1], dt.float32)
            nc.vector.memset(eps_sb[:], EPS)

            xq_t, xo_t, ab_t = [], [], []
            for cc in range(CC):
                xqt = hq_pool.tile([P, NQ], qk_dt, name=f"hq_{cc}", tag=f"hq_{cc}")
                xot = ho_pool.tile([P, NQ], qk_dt, name=f"ho_{cc}", tag=f"ho_{cc}")
                for hcol in range(2):
                    cs = slice(hcol * NQ // 2, (hcol + 1) * NQ // 2)
                    nc.sync.dma_start(out=xqt[:, cs],
                                      in_=xq_ext[cc * P:(cc + 1) * P, cs].bitcast(qk_dt))
                    nc.sync.dma_start(out=xot[:, cs],
                                      in_=xo_ext[cc * P:(cc + 1) * P, cs].bitcast(qk_dt))
                xq_t.append(xqt)
                xo_t.append(xot)

            for cc in range(CC):
                xqf = xq_t[cc][:].bitcast(dt.float32)
                xof = xo_t[cc][:].bitcast(dt.float32)
                stats = stat_pool.tile([P, 8, 6], dt.float32, name=f"st_{cc}", tag="st", bufs=2)
                for j in range(4):
                    nc.vector.bn_stats(out=stats[:, j, :], in_=xqf[:, j * 512:(j + 1) * 512])
                for j in range(4):
                    nc.vector.bn_stats(out=stats[:, 4 + j, :], in_=xof[:, j * 512:(j + 1) * 512])
                mv = stat_pool.tile([P, 2], dt.float32, name=f"mv_{cc}", tag="mv", bufs=2)
                nc.vector.bn_aggr(out=mv[:], in_=stats[:])
                # mv[:,1] := var + mean^2  (per-channel second moment)
                sq = stat_pool.tile([P, 1], dt.float32, name=f"sq_{cc}", tag="sq", bufs=2)
                nc.vector.tensor_mul(sq[:], mv[:, 0:1], mv[:, 0:1])
                nc.vector.tensor_add(mv[:, 1:2], mv[:, 1:2], sq[:])

                # this chunk's 8 groups: [32, 2] = sum_c gsel[c,g] * mv[c,:]
                gps = ps_small.tile([NUM_GROUPS, 2], dt.float32, tag="gps", bufs=1)
                nc.tensor.matmul(gps[:], gsel_sb[:, cc, :], mv[:], start=True, stop=True)
                g_sb = small.tile([NUM_GROUPS, 2], dt.float32, name=f"g_{cc}", tag="g", bufs=2)
                nc.scalar.copy(g_sb[:], gps[:])
                # var_g = E[x^2] - mean^2 ; rstd = exp(-0.5*ln(var+eps))
                # (ln+exp live in one ACT table set; sqrt would force a table swap)
                gm2 = small.tile([NUM_GROUPS, 1], dt.float32, name=f"gm2_{cc}", tag="gm2", bufs=2)
                nc.vector.tensor_mul(gm2[:], g_sb[:, 0:1], g_sb[:, 0:1])
                grp = small.tile([NUM_GROUPS, 2], dt.float32, name=f"grp_{cc}", tag="grp", bufs=2)
                nc.vector.tensor_copy(grp[:, 0:1], g_sb[:, 0:1])
                varg = small.tile([NUM_GROUPS, 1], dt.float32, name=f"varg_{cc}", tag="varg", bufs=2)
                nc.vector.tensor_sub(varg[:], g_sb[:, 1:2], gm2[:])
                lng = small.tile([NUM_GROUPS, 1], dt.float32, name=f"lng_{cc}", tag="lng", bufs=2)
                nc.scalar.activation(lng[:], varg[:], mybir.ActivationFunctionType.Ln,
                                     bias=eps_sb[:], scale=1.0)
                nc.scalar.activation(grp[:, 1:2], lng[:], mybir.ActivationFunctionType.Exp,
                                     bias=0.0, scale=-0.5)

                # broadcast (mean, rstd) to this chunk's channels; GN affine fold:
                # a = gnw*rstd ; b = gnb - mean*a
                pcs = ps_small.tile([P, 2], dt.float32, tag="pcs", bufs=1)
                nc.tensor.matmul(pcs[:], esel_sb[:, cc * P:(cc + 1) * P], grp[:],
                                 start=True, stop=True)
                pc = small.tile([P, 2], dt.float32, name=f"pc_{cc}", tag="pc", bufs=2)
                nc.scalar.copy(pc[:], pcs[:])
                ab = affine.tile([P, 2], dt.float32, name=f"ab_{cc}", tag=f"ab_{cc}")
                nc.vector.tensor_mul(ab[:, 0:1], gn_ab_sb[:, cc, 0:1], pc[:, 1:2])
                t0 = small.tile([P, 1], dt.float32, name=f"t0_{cc}", tag="t0", bufs=2)
                nc.vector.tensor_mul(t0[:], pc[:, 0:1], ab[:, 0:1])
                nc.vector.tensor_sub(ab[:, 1:2], gn_ab_sb[:, cc, 1:2], t0[:])
                ab_t.append(ab)

            gn_stack.close()

            def x_cols(cc, col0, width):
                """x[cc][:, col0:col0+width] in the core-local order [xq | xo]."""
                if col0 < NQ:
                    return xq_t[cc][:, col0:col0 + width]
                return xo_t[cc][:, col0 - NQ:col0 - NQ + width]

            # ---------------- Phase 2: K / V / Q convs (from x directly) --------
            conv_ps_stack = ExitStack()
            ps_conv = conv_ps_stack.enter_context(
                tc.tile_pool(name="ps_conv", bufs=4, space="PSUM", side="right"))

            def fold_weight(wT_sb, bias_col):
                """Per channel chunk: bias' += W_cc^T b_cc, then scale W_cc in
                place (W'[c, o] = W[c, o] * a[c]) — each chunk gated only on its
                own group stats."""
                bacc_sb = affine.tile([P, CC], dt.float32, name=f"bacc_{bias_col}",
                                      tag=f"bacc_{bias_col}")
                for cc in range(CC):
                    for oc in range(CC):
                        bps = ps_conv.tile([P, 1], dt.float32, tag="bps", bufs=2)
                        nc.tensor.matmul(bps[:],
                                         wT_sb[:, cc, oc * P:(oc + 1) * P].bitcast(dt.float32),
                                         ab_t[cc][:, 1:2], start=True, stop=True)
                        if cc == 0:
                            nc.vector.tensor_copy(bacc_sb[:, oc:oc + 1], bps[:])
                        else:
                            nc.vector.tensor_add(bacc_sb[:, oc:oc + 1],
                                                 bacc_sb[:, oc:oc + 1], bps[:])
                    nc.vector.tensor_scalar_mul(wT_sb[:, cc, :],
                                                wT_sb[:, cc, :].bitcast(dt.float32),
                                                ab_t[cc][:, 0:1])
                for oc in range(CC):
                    nc.vector.tensor_add(bx_sb[:, oc, bias_col:bias_col + 1],
                                         bacc_sb[:, oc:oc + 1],
                                         biases_sb[:, oc, bias_col:bias_col + 1])

            wk_stack = ExitStack()
            wk_pool = wk_stack.enter_context(tc.tile_pool(name="wk_pool", bufs=1, side="right"))
            wkT_sb = wk_pool.tile([P, CC, C], qk_dt)
            for cc in range(CC):
                nc.sync.dma_start(out=wkT_sb[:, cc, :],
                                  in_=wkT_ext[cc * P:(cc + 1) * P, :].bitcast(qk_dt))
            fold_weight(wkT_sb, 1)

            k_t = [k_pool.tile([P, N], qk_dt, name=f"k_{oc}", tag=f"k_{oc}") for oc in range(CC)]
            for oc in range(CC):
                for ncol in range(N // 512):
                    ps = ps_conv.tile([P, 512], dt.float32, tag="conv", bufs=4)
                    for cc in range(CC):
                        nc.tensor.matmul(ps[:], wkT_sb[:, cc, oc * P:(oc + 1) * P],
                                         x_cols(cc, ncol * 512, 512),
                                         start=(cc == 0), stop=(cc == CC - 1))
                    if ncol % 2 == 0:
                        nc.vector.tensor_scalar(
                            out=k_t[oc][:, ncol * 512:(ncol + 1) * 512], in0=ps[:],
                            scalar1=bx_sb[:, oc, 1:2], scalar2=None,
                            op0=mybir.AluOpType.add)
                    else:
                        nc.scalar.activation(
                            out=k_t[oc][:, ncol * 512:(ncol + 1) * 512], in_=ps[:],
                            func=mybir.ActivationFunctionType.Identity,
                            bias=bx_sb[:, oc, 1:2], scale=1.0)
            wk_stack.close()

            wv_stack = ExitStack()
            wv_pool = wv_stack.enter_context(tc.tile_pool(name="wv_pool", bufs=1, side="right"))
            wvT_sb = wv_pool.tile([P, CC, C], qk_dt)
            for cc in range(CC):
                nc.sync.dma_start(out=wvT_sb[:, cc, :],
                                  in_=wvT_ext[cc * P:(cc + 1) * P, :].bitcast(qk_dt))
            fold_weight(wvT_sb, 2)

            # vT[pix, c_out] = x^T wv'T  (+bias_v' folded into attn output later)
            vT_sb = vT_pool.tile([P, N // P, C], dt.bfloat16)
            for pc in range(N // P):
                ps = ps_conv.tile([P, C], dt.float32, tag="conv", bufs=4)
                for cc in range(CC):
                    nc.tensor.matmul(ps[:], x_cols(cc, pc * P, P), wvT_sb[:, cc, :],
                                     start=(cc == 0), stop=(cc == CC - 1))
                nc.scalar.copy(vT_sb[:, pc, :], ps[:])
            wv_stack.close()

            wq_stack = ExitStack()
            wq_pool = wq_stack.enter_context(tc.tile_pool(name="wq_pool", bufs=1, side="right"))
            wqT_sb = wq_pool.tile([P, CC, C], qk_dt)
            for cc in range(CC):
                nc.sync.dma_start(out=wqT_sb[:, cc, :],
                                  in_=wqT_ext[cc * P:(cc + 1) * P, :].bitcast(qk_dt))
            fold_weight(wqT_sb, 0)

            q_t = [q_pool.tile([P, NQ], qk_dt, name=f"q_{oc}", tag=f"q_{oc}") for oc in range(CC)]
            for oc in range(CC):
                for ncol in range(NQ // 512):
                    ps = ps_conv.tile([P, 512], dt.float32, tag="conv", bufs=4)
                    for cc in range(CC):
                        nc.tensor.matmul(ps[:], wqT_sb[:, cc, oc * P:(oc + 1) * P],
                                         xq_t[cc][:, ncol * 512:(ncol + 1) * 512],
                                         start=(cc == 0), stop=(cc == CC - 1))
                    if ncol % 2 == 0:
                        nc.vector.tensor_scalar(
                            out=q_t[oc][:, ncol * 512:(ncol + 1) * 512], in0=ps[:],
                            scalar1=bx_sb[:, oc, 0:1], scalar2=None,
                            op0=mybir.AluOpType.add)
                    else:
                        nc.scalar.activation(
                            out=q_t[oc][:, ncol * 512:(ncol + 1) * 512], in_=ps[:],
                            func=mybir.ActivationFunctionType.Identity,
                            bias=bx_sb[:, oc, 0:1], scale=1.0)
            wq_stack.close()
            conv_ps_stack.close()
            af_stack.close()
            ho_stack.close()
            hq_stack.close()

            # ---------------- Phase 3: attention ----------------
            at_stack = ExitStack()
            at = at_stack.enter_context(tc.tile_pool(name="at", bufs=2, side="left"))
            wT_pool = at_stack.enter_context(tc.tile_pool(name="wT_pool", bufs=1, side="left"))
            out_pool = at_stack.enter_context(tc.tile_pool(name="out_pool", bufs=2, side="left"))
            ps_sc = at_stack.enter_context(
                tc.tile_pool(name="ps_sc", bufs=2, space="PSUM", side="left"))
            ps_tp = at_stack.enter_context(
                tc.tile_pool(name="ps_tp", bufs=2, space="PSUM", side="left"))
            ps_at = at_stack.enter_context(
                tc.tile_pool(name="ps_at", bufs=2, space="PSUM", side="left"))

            ident = at.tile([P, P], dt.bfloat16, tag="ident", bufs=1)
            make_identity(nc, ident[:])
            woT_sb = at.tile([P, CC, C], dt.bfloat16, tag="woT", bufs=1)
            nc.gpsimd.dma_start(out=woT_sb[:], in_=woT_ext.rearrange("(c p) o -> p c o", p=P))

            for qg in range(QCH // 4):  # groups of 4 query chunks (512 queries)
                wT_sb = wT_pool.tile([P, N // P, 512], dt.bfloat16, tag="wT")
                for qi4 in range(4):
                    qi = qg * 4 + qi4
                    # --- scores + online softmax over 4 quarters of k ---
                    e_q = at.tile([P, NQW, NKQ], dt.bfloat16, tag="e", bufs=2)
                    mq = at.tile([P, NQW], dt.float32, tag="mq")
                    sq = at.tile([P, NQW], dt.float32, tag="sq")
                    bias_t = at.tile([P, NQW], dt.float32, tag="bias")
                    for w in range(NQW):
                        ps = ps_sc.tile([P, NKQ], dt.float32, tag="sc", bufs=2)
                        for half in range(2):
                            col0 = w * NKQ + half * 512
                            for cc in range(CC):
                                nc.tensor.matmul(
                                    ps[:, half * 512:(half + 1) * 512],
                                    q_t[cc][:, qi * P:(qi + 1) * P],
                                    k_t[cc][:, col0:col0 + 512],
                                    start=(cc == 0), stop=(cc == CC - 1))
                        nc.vector.reduce_max(out=mq[:, w:w + 1], in_=ps[:],
                                             axis=mybir.AxisListType.X)
                        nc.vector.tensor_scalar_mul(bias_t[:, w:w + 1], mq[:, w:w + 1],
                                                    -SCALE)
                        nc.scalar.activation(
                            out=e_q[:, w, :], in_=ps[:],
                            func=mybir.ActivationFunctionType.Exp,
                            bias=bias_t[:, w:w + 1], scale=SCALE,
                            accum_out=sq[:, w:w + 1])
                    # combine quarters: m = max_w mq ; alpha_w = exp(SCALE*(mq-m))/s
                    m_t = at.tile([P, 1], dt.float32, tag="m")
                    nc.vector.reduce_max(out=m_t[:], in_=mq[:], axis=mybir.AxisListType.X)
                    mb = at.tile([P, 1], dt.float32, tag="mb")
                    nc.vector.tensor_scalar_mul(mb[:], m_t[:], -SCALE)
                    beta = at.tile([P, NQW], dt.float32, tag="beta")
                    nc.scalar.activation(out=beta[:], in_=mq[:],
                                         func=mybir.ActivationFunctionType.Exp,
                                         bias=mb[:], scale=SCALE)
                    sb_t = at.tile([P, NQW], dt.float32, tag="sbt")
                    nc.vector.tensor_mul(sb_t[:], sq[:], beta[:])
                    s_t = at.tile([P, 1], dt.float32, tag="s")
                    nc.vector.reduce_sum(out=s_t[:], in_=sb_t[:], axis=mybir.AxisListType.X)
                    rs = at.tile([P, 1], dt.float32, tag="rs")
                    nc.vector.reciprocal(rs[:], s_t[:])
                    alpha = at.tile([P, NQW], dt.float32, tag="alpha")
                    nc.vector.tensor_scalar_mul(alpha[:], beta[:], rs[:])
                    # normalize e, then transpose into wT columns for this chunk
                    for w in range(NQW):
                        nc.vector.tensor_scalar_mul(e_q[:, w, :], e_q[:, w, :],
                                                    alpha[:, w:w + 1])
                    for w in range(NQW):
                        tp = ps_tp.tile([P, 8, P], dt.bfloat16, tag="tp", bufs=2)
                        for j in range(8):
                            nc.tensor.transpose(
                                tp[:, j, :], e_q[:, w, j * P:(j + 1) * P], ident[:])
                        dst = wT_sb[:, w * 8:w * 8 + 8, qi4 * P:(qi4 + 1) * P]
                        if w % 2 == 0:
                            nc.scalar.copy(dst, tp[:])
                        else:
                            nc.vector.tensor_copy(dst, tp[:])

                # --- attn = v @ weights^T for this 512-query group ---
                # kc-outer in oc-pairs: each wT[kc] slice is fully consumed early,
                # letting the next group's transposes start before this group ends.
                attn_sb = at.tile([P, CC, 512], dt.bfloat16, tag="attn")
                for oc0 in (0, 2):
                    ps_pair = [ps_at.tile([P, 512], dt.float32, tag="at", bufs=2,
                                          name=f"at_ps_{oc0}_{j}")
                               for j in range(2)]
                    for kc in range(N // P):
                        for j in range(2):
                            oc = oc0 + j
                            nc.tensor.matmul(ps_pair[j][:],
                                             vT_sb[:, kc, oc * P:(oc + 1) * P],
                                             wT_sb[:, kc, :],
                                             start=(kc == 0), stop=(kc == N // P - 1))
                    for j in range(2):
                        oc = oc0 + j
                        # + folded v bias (softmax weights sum to 1, so +b[c] is exact)
                        nc.scalar.activation(out=attn_sb[:, oc, :], in_=ps_pair[j][:],
                                             func=mybir.ActivationFunctionType.Identity,
                                             bias=bx_sb[:, oc, 2:3], scale=1.0)

                # --- out = wo @ attn + bo + xq ---
                for oc in range(CC):
                    ps = ps_at.tile([P, 512], dt.float32, tag="at", bufs=2)
                    for cc in range(CC):
                        nc.tensor.matmul(ps[:], woT_sb[:, cc, oc * P:(oc + 1) * P],
                                         attn_sb[:, cc, :],
                                         start=(cc == 0), stop=(cc == CC - 1))
                    xq_sb = out_pool.tile([P, 512], dt.float32, tag="xq", bufs=2)
                    nc.sync.dma_start(out=xq_sb[:],
                                      in_=xq_ext[oc * P:(oc + 1) * P, qg * 512:(qg + 1) * 512])
                    o_sb = out_pool.tile([P, 512], dt.float32, tag="o", bufs=2)
                    nc.scalar.activation(out=o_sb[:], in_=ps[:],
                                         func=mybir.ActivationFunctionType.Identity,
                                         bias=biases_sb[:, oc, 3:4], scale=1.0)
                    nc.gpsimd.tensor_add(o_sb[:], o_sb[:], xq_sb[:])
                    nc.sync.dma_start(
                        out=out_ext[oc * P:(oc + 1) * P, qg * 512:(qg + 1) * 512],
                        in_=o_sb[:])
            at_stack.close()
            top.close()

    # Force every activation onto the natural_log_exp_and_others table set so
    # the kernel never pays a mid-run ACT table swap (~2.7us each).
    import concourse.bacc as bacc_mod
    orig_tables = bacc_mod.get_activation_tables

    def one_set_tables(arch):
        t = dict(orig_tables(arch))
        return {name: (funcs if name == "natural_log_exp_and_others" else frozenset())
                for name, funcs in t.items()}

    bacc_mod.get_activation_tables = one_set_tables
    try:
        nc.compile()
    finally:
        bacc_mod.get_activation_tables = orig_tables
    return nc


def _get_nc(qk_mode, repeats=1):
    key = (qk_mode, repeats)
    if key not in _CACHE:
        _CACHE[key] = _build(qk_mode, repeats)
    return _CACHE[key]


def kernel(x, gn_weight, gn_bias, wq, bq, wk, bk, wv, bv, wo, bo):
    from concourse.bass_utils import run_bass_kernel_spmd

    nc = _get_nc(QK_MODE)

    x = np.asarray(x, dtype=np.float32)
    f32 = lambda a: np.ascontiguousarray(np.asarray(a, dtype=np.float32))

    wqT = f32(np.asarray(wq, dtype=np.float32).T)
    wkT = f32(np.asarray(wk, dtype=np.float32).T)
    wvT = f32(np.asarray(wv, dtype=np.float32).T)
    woT = f32(np.asarray(wo, dtype=np.float32).T)
    biases = f32(np.stack([bq, bk, bv, bo], axis=1))        # [C, 4]
    gn_ab = f32(np.stack([gn_weight, gn_bias], axis=1))     # [C, 2]

    gsel = np.zeros((C, NUM_GROUPS), dtype=np.float32)
    gsel[np.arange(C), np.arange(C) // GSIZE] = 1.0 / GSIZE
    esel = np.zeros((NUM_GROUPS, C), dtype=np.float32)
    esel[np.arange(C) // GSIZE, np.arange(C)] = 1.0

    in_maps = []
    for core in range(8):
        b, half = core // 2, core % 2
        xb = x[b].reshape(C, N)
        xqb = f32(xb[:, half * NQ:(half + 1) * NQ])
        xob = f32(xb[:, (1 - half) * NQ:(2 - half) * NQ])
        in_maps.append({
            "xq": xqb, "xo": xob,
            "wqT": wqT, "wkT": wkT, "wvT": wvT, "woT": woT,
            "biases": biases, "gn_ab": gn_ab, "gsel": gsel, "esel": esel,
        })

    import os
    trace = bool(os.environ.get("BASS_TRACE"))
    res = run_bass_kernel_spmd(nc, in_maps, core_ids=list(range(8)),
                               trace=trace)
    global _LAST
    _LAST = res

    out = np.empty((B, C, N), dtype=np.float32)
    for core in range(8):
        b, half = core // 2, core % 2
        out[b, :, half * NQ:(half + 1) * NQ] = res.results[core]["out"]
    return out.reshape(B, C, H, W)



# revision 3
# speedup vs baseline: 1.1265x; 1.1265x over previous
"""AttnBlock (GroupNorm -> single-head attention over 64x64 pixels -> out conv
-> residual) on 8 Trainium2 NeuronCores.

Sharding: data parallel over batch (B=4) x 2-way split of the query-pixel axis
=> 8 cores, no collectives.  Each core receives its batch element's pixels as
two inputs: xq (its 2048 query columns) and xo (the other 2048 columns).  K and
the fused V·out-conv ("u") are computed over all 4096 pixels in the core-local
order [xq | xo].

Key structure (v2):
  * out-conv is fused into v on the host: wu = wo @ wv, so the attention
    matmul directly produces the final (pre-residual) output.
  * attention output is computed TRANSPOSED ([query, channel]) so each
    128-query chunk's weight matrix is an 8KB tile -> fine-grained pipeline.
  * the e-matrix [query, key] -> [key, query] transpose runs on the DMA XBAR
    (dma_start(transpose=True)), not the PE / ACT / DVE engines.
  * the k-conv bias is dropped: a per-key-channel bias shifts every score of
    a query row by the same amount -> softmax invariant.

All shapes hardcoded: B=4, C=512, H=W=64, N=4096, 32 groups.
"""

import numpy as np

B, C, H, W = 4, 512, 64, 64
N = H * W              # 4096 pixels
NQ = N // 2            # 2048 query pixels per core
NUM_GROUPS = 32
GSIZE = C // NUM_GROUPS  # 16 channels per group
EPS = 1e-6
SCALE = float(C) ** 0.5  # reference multiplies scores by sqrt(C)

P = 128                # partitions
CC = C // P            # 4 channel chunks
QCH = NQ // P          # 16 query chunks per core
NKQ = 1024             # k-columns per score quarter
NQW = N // NKQ         # 4 quarters per query chunk
KCH = N // P           # 32 key chunks

# "fp32r" = fast reduced-precision fp32 matmul for the q/k/scores path;
# "fp32" = full precision (4x slower on PE for the scores matmuls).
QK_MODE = "fp32r"

_CACHE = {}
_LAST = None


def _build(qk_mode, repeats=1):
    from contextlib import ExitStack

    import concourse.bacc as bacc
    import concourse.tile as tile
    from concourse import mybir
    from concourse.masks import make_identity

    dt = mybir.dt
    qk_dt = dt.float32r if qk_mode == "fp32r" else dt.float32

    nc = bacc.Bacc()
    xq_ext = nc.declare_dram_parameter("xq", [C, NQ], dt.float32, isOutput=False)
    xo_ext = nc.declare_dram_parameter("xo", [C, NQ], dt.float32, isOutput=False)
    xqT_ext = nc.declare_dram_parameter("xqT", [NQ, C], dt.float32, isOutput=False)
    wqT_ext = nc.declare_dram_parameter("wqT", [C, C], dt.float32, isOutput=False)
    wkT_ext = nc.declare_dram_parameter("wkT", [C, C], dt.float32, isOutput=False)
    wuT_ext = nc.declare_dram_parameter("wuT", [C, C], dt.float32, isOutput=False)
    biases_ext = nc.declare_dram_parameter("biases", [C, 2], dt.float32, isOutput=False)
    gn_ab_ext = nc.declare_dram_parameter("gn_ab", [C, 2], dt.float32, isOutput=False)
    gsel_ext = nc.declare_dram_parameter("gsel", [C, NUM_GROUPS], dt.float32, isOutput=False)
    esel_ext = nc.declare_dram_parameter("esel", [NUM_GROUPS, C], dt.float32, isOutput=False)
    out_ext = nc.declare_dram_parameter("out", [NQ, C], dt.float32, isOutput=True)

    with tile.TileContext(nc) as tc:
        # LEFT side: long-lived pools (whole kernel / attention phase).
        # RIGHT side: phase-scoped pools (GN scratch, conv weights, x).
        for _rep in range(repeats):
            top = ExitStack()
            const = top.enter_context(tc.tile_pool(name="const", bufs=1, side="left"))
            biases_sb = const.tile([P, CC, 2], dt.float32)  # [:, :, 0..1] = bq, bu
            nc.sync.dma_start(out=biases_sb[:], in_=biases_ext.rearrange("(c p) k -> p c k", p=P))
            k_pool = top.enter_context(tc.tile_pool(name="k_pool", bufs=1, side="left"))
            u_pool = top.enter_context(tc.tile_pool(name="u_pool", bufs=1, side="left"))
            q_pool = top.enter_context(tc.tile_pool(name="q_pool", bufs=1, side="left"))

            # ---------------- Phase 1: GroupNorm stats (folded into convs) ------
            # GroupNorm h = a*x + b is folded into the conv weights:
            #   W' = W @ diag(a),  bias' = W @ b + bias
            # so K/Q/U are computed directly from x and h never materializes.
            bx_sb = const.tile([P, CC, 2], dt.float32)  # folded conv biases q, u
            cb_row = const.tile([P, C], dt.float32)     # u bias broadcast per query row
            ident = const.tile([P, P], dt.float32)
            make_identity(nc, ident[:])

            hq_stack = ExitStack()
            hq_pool = hq_stack.enter_context(tc.tile_pool(name="hq_pool", bufs=1, side="right"))
            ho_stack = ExitStack()
            ho_pool = ho_stack.enter_context(tc.tile_pool(name="ho_pool", bufs=1, side="right"))
            af_stack = ExitStack()
            affine = af_stack.enter_context(tc.tile_pool(name="affine", bufs=1, side="right"))
            gn_stack = ExitStack()
            stat_pool = gn_stack.enter_context(tc.tile_pool(name="stat_pool", bufs=1, side="right"))
            small = gn_stack.enter_context(tc.tile_pool(name="small", bufs=1, side="right"))
            ps_small = gn_stack.enter_context(
                tc.tile_pool(name="ps_small", bufs=1, space="PSUM", side="right"))

            gsel_sb = small.tile([P, CC, NUM_GROUPS], dt.float32)
            nc.sync.dma_start(out=gsel_sb[:], in_=gsel_ext.rearrange("(c p) g -> p c g", p=P))
            esel_sb = small.tile([NUM_GROUPS, C], dt.float32)
            nc.sync.dma_start(out=esel_sb[:], in_=esel_ext[:])
            gn_ab_sb = small.tile([P, CC, 2], dt.float32)
            nc.sync.dma_start(out=gn_ab_sb[:], in_=gn_ab_ext.rearrange("(c p) k -> p c k", p=P))
            eps_sb = small.tile([NUM_GROUPS, 1], dt.float32)
            nc.vector.memset(eps_sb[:], EPS)

            xq_t, xo_t, ab_t = [], [], []
            for cc in range(CC):
                xqt = hq_pool.tile([P, NQ], qk_dt, name=f"hq_{cc}", tag=f"hq_{cc}")
                xot = ho_pool.tile([P, NQ], qk_dt, name=f"ho_{cc}", tag=f"ho_{cc}")
                for hcol in range(2):
                    cs = slice(hcol * NQ // 2, (hcol + 1) * NQ // 2)
                    nc.sync.dma_start(out=xqt[:, cs],
                                      in_=xq_ext[cc * P:(cc + 1) * P, cs].bitcast(qk_dt))
                    nc.sync.dma_start(out=xot[:, cs],
                                      in_=xo_ext[cc * P:(cc + 1) * P, cs].bitcast(qk_dt))
                xq_t.append(xqt)
                xo_t.append(xot)

            for cc in range(CC):
                xqf = xq_t[cc][:].bitcast(dt.float32)
                xof = xo_t[cc][:].bitcast(dt.float32)
                stats = stat_pool.tile([P, 8, 6], dt.float32, name=f"st_{cc}", tag="st", bufs=2)
                for j in range(4):
                    nc.vector.bn_stats(out=stats[:, j, :], in_=xqf[:, j * 512:(j + 1) * 512])
                for j in range(4):
                    nc.vector.bn_stats(out=stats[:, 4 + j, :], in_=xof[:, j * 512:(j + 1) * 512])
                mv = stat_pool.tile([P, 2], dt.float32, name=f"mv_{cc}", tag="mv", bufs=2)
                nc.vector.bn_aggr(out=mv[:], in_=stats[:])
                # mv[:,1] := var + mean^2  (per-channel second moment)
                sq = stat_pool.tile([P, 1], dt.float32, name=f"sq_{cc}", tag="sq", bufs=2)
                nc.vector.tensor_mul(sq[:], mv[:, 0:1], mv[:, 0:1])
                nc.vector.tensor_add(mv[:, 1:2], mv[:, 1:2], sq[:])

                # this chunk's 8 groups: [32, 2] = sum_c gsel[c,g] * mv[c,:]
                gps = ps_small.tile([NUM_GROUPS, 2], dt.float32, tag="gps", bufs=1)
                nc.tensor.matmul(gps[:], gsel_sb[:, cc, :], mv[:], start=True, stop=True)
                g_sb = small.tile([NUM_GROUPS, 2], dt.float32, name=f"g_{cc}", tag="g", bufs=2)
                nc.scalar.copy(g_sb[:], gps[:])
                # var_g = E[x^2] - mean^2 ; rstd = exp(-0.5*ln(var+eps))
                # (ln+exp live in one ACT table set; sqrt would force a table swap)
                gm2 = small.tile([NUM_GROUPS, 1], dt.float32, name=f"gm2_{cc}", tag="gm2", bufs=2)
                nc.vector.tensor_mul(gm2[:], g_sb[:, 0:1], g_sb[:, 0:1])
                grp = small.tile([NUM_GROUPS, 2], dt.float32, name=f"grp_{cc}", tag="grp", bufs=2)
                nc.vector.tensor_copy(grp[:, 0:1], g_sb[:, 0:1])
                varg = small.tile([NUM_GROUPS, 1], dt.float32, name=f"varg_{cc}", tag="varg", bufs=2)
                nc.vector.tensor_sub(varg[:], g_sb[:, 1:2], gm2[:])
                lng = small.tile([NUM_GROUPS, 1], dt.float32, name=f"lng_{cc}", tag="lng", bufs=2)
                nc.scalar.activation(lng[:], varg[:], mybir.ActivationFunctionType.Ln,
                                     bias=eps_sb[:], scale=1.0)
                nc.scalar.activation(grp[:, 1:2], lng[:], mybir.ActivationFunctionType.Exp,
                                     bias=0.0, scale=-0.5)

                # broadcast (mean, rstd) to this chunk's channels; GN affine fold:
                # a = gnw*rstd ; b = gnb - mean*a
                pcs = ps_small.tile([P, 2], dt.float32, tag="pcs", bufs=1)
                nc.tensor.matmul(pcs[:], esel_sb[:, cc * P:(cc + 1) * P], grp[:],
                                 start=True, stop=True)
                pc = small.tile([P, 2], dt.float32, name=f"pc_{cc}", tag="pc", bufs=2)
                nc.scalar.copy(pc[:], pcs[:])
                ab = affine.tile([P, 2], dt.float32, name=f"ab_{cc}", tag=f"ab_{cc}")
                nc.vector.tensor_mul(ab[:, 0:1], gn_ab_sb[:, cc, 0:1], pc[:, 1:2])
                t0 = small.tile([P, 1], dt.float32, name=f"t0_{cc}", tag="t0", bufs=2)
                nc.vector.tensor_mul(t0[:], pc[:, 0:1], ab[:, 0:1])
                nc.vector.tensor_sub(ab[:, 1:2], gn_ab_sb[:, cc, 1:2], t0[:])
                ab_t.append(ab)

            gn_stack.close()

            def x_cols(cc, col0, width):
                """x[cc][:, col0:col0+width] in the core-local order [xq | xo]."""
                if col0 < NQ:
                    return xq_t[cc][:, col0:col0 + width]
                return xo_t[cc][:, col0 - NQ:col0 - NQ + width]

            # ---------------- Phase 2: K / Q / U convs (from x directly) --------
            conv_ps_stack = ExitStack()
            ps_conv = conv_ps_stack.enter_context(
                tc.tile_pool(name="ps_conv", bufs=5, space="PSUM", side="right"))

            def fold_weight(wT_sb, bias_col):
                """bias' = sum_cc W_cc^T b_cc (+host bias), accumulated in
                PSUM; then scale W_cc in place (W'[c, o] = W[c, o] * a[c])."""
                if bias_col is not None:
                    for oc in range(CC):
                        bps = ps_conv.tile([P, 1], dt.float32, tag="bps", bufs=1)
                        for cc in range(CC):
                            nc.tensor.matmul(bps[:],
                                             wT_sb[:, cc, oc * P:(oc + 1) * P].bitcast(dt.float32),
                                             ab_t[cc][:, 1:2],
                                             start=(cc == 0), stop=(cc == CC - 1))
                        nc.vector.tensor_add(bx_sb[:, oc, bias_col:bias_col + 1],
                                             bps[:],
                                             biases_sb[:, oc, bias_col:bias_col + 1])
                for cc in range(CC):
                    nc.vector.tensor_scalar_mul(wT_sb[:, cc, :],
                                                wT_sb[:, cc, :].bitcast(dt.float32),
                                                ab_t[cc][:, 0:1])

            # --- K conv (no bias: per-key-channel bias is softmax-invariant) ---
            wk_stack = ExitStack()
            wk_pool = wk_stack.enter_context(tc.tile_pool(name="wk_pool", bufs=1, side="right"))
            wkT_sb = wk_pool.tile([P, CC, C], qk_dt)
            for cc in range(CC):
                nc.sync.dma_start(out=wkT_sb[:, cc, :],
                                  in_=wkT_ext[cc * P:(cc + 1) * P, :].bitcast(qk_dt))
            fold_weight(wkT_sb, None)

            k_t = [k_pool.tile([P, N], qk_dt, name=f"k_{oc}", tag=f"k_{oc}") for oc in range(CC)]
            for oc in range(CC):
                for ncol in range(N // 512):
                    ps = ps_conv.tile([P, 512], dt.float32, tag="conv", bufs=5)
                    for cc in range(CC):
                        nc.tensor.matmul(ps[:], wkT_sb[:, cc, oc * P:(oc + 1) * P],
                                         x_cols(cc, ncol * 512, 512),
                                         start=(cc == 0), stop=(cc == CC - 1))
                    dst = k_t[oc][:, ncol * 512:(ncol + 1) * 512]
                    if ncol % 2 == 0:
                        nc.vector.tensor_copy(dst, ps[:])
                    else:
                        nc.scalar.copy(dst, ps[:])
            wk_stack.close()

            # --- Q conv (bias kept: shifts scores per-key -> not invariant) ---
            wq_stack = ExitStack()
            wq_pool = wq_stack.enter_context(tc.tile_pool(name="wq_pool", bufs=1, side="right"))
            wqT_sb = wq_pool.tile([P, CC, C], qk_dt)
            for cc in range(CC):
                nc.sync.dma_start(out=wqT_sb[:, cc, :],
                                  in_=wqT_ext[cc * P:(cc + 1) * P, :].bitcast(qk_dt))
            fold_weight(wqT_sb, 0)

            q_t = [q_pool.tile([P, NQ], qk_dt, name=f"q_{oc}", tag=f"q_{oc}") for oc in range(CC)]
            for oc in range(CC):
                for ncol in range(NQ // 512):
                    ps = ps_conv.tile([P, 512], dt.float32, tag="conv", bufs=5)
                    for cc in range(CC):
                        nc.tensor.matmul(ps[:], wqT_sb[:, cc, oc * P:(oc + 1) * P],
                                         xq_t[cc][:, ncol * 512:(ncol + 1) * 512],
                                         start=(cc == 0), stop=(cc == CC - 1))
                    dst = q_t[oc][:, ncol * 512:(ncol + 1) * 512]
                    if ncol % 2 == 0:
                        nc.vector.tensor_scalar(
                            out=dst, in0=ps[:],
                            scalar1=bx_sb[:, oc, 0:1], scalar2=None,
                            op0=mybir.AluOpType.add)
                    else:
                        nc.scalar.activation(
                            out=dst, in_=ps[:],
                            func=mybir.ActivationFunctionType.Identity,
                            bias=bx_sb[:, oc, 0:1], scale=1.0)
            wq_stack.close()

            # --- U conv: u = (wo@wv)' @ x, [pixel, channel] layout, bf16.
            # The u bias (wu@b + wo@bv + bo) is NOT applied here: softmax
            # weights sum to 1, so it is added once at the end via cb_row.
            wu_stack = ExitStack()
            wu_pool = wu_stack.enter_context(tc.tile_pool(name="wu_pool", bufs=1, side="right"))
            wuT_sb = wu_pool.tile([P, CC, C], qk_dt)
            for cc in range(CC):
                nc.sync.dma_start(out=wuT_sb[:, cc, :],
                                  in_=wuT_ext[cc * P:(cc + 1) * P, :].bitcast(qk_dt))
            fold_weight(wuT_sb, 1)

            # --- cb_row: broadcast the folded u bias to all 128 query rows.
            # bx_sb[:, oc, 1] is [128, 1] per chunk; PE-transpose each to
            # [1, 128] then broadcast-matmul with a ones column.
            cb_stack = ExitStack()
            cb_pool = cb_stack.enter_context(tc.tile_pool(name="cb_pool", bufs=1, side="right"))
            ps_cb = cb_stack.enter_context(
                tc.tile_pool(name="ps_cb", bufs=1, space="PSUM", side="right"))
            tp1 = ps_cb.tile([1, CC, P], dt.float32, tag="tp1")
            for oc in range(CC):
                nc.tensor.transpose(tp1[:, oc, :], bx_sb[:, oc, 1:2], ident[:])
            cb1 = cb_pool.tile([1, CC, P], dt.float32)
            nc.scalar.copy(cb1[:], tp1[:])
            ones1 = cb_pool.tile([1, P], dt.float32)
            nc.vector.memset(ones1[:], 1.0)
            cb_ps = ps_cb.tile([P, C], dt.float32, tag="cbps")
            for oc in range(CC):
                nc.tensor.matmul(cb_ps[:, oc * P:(oc + 1) * P], ones1[:],
                                 cb1[:, oc, :], start=True, stop=True)
            nc.scalar.copy(cb_row[:], cb_ps[:])
            cb_stack.close()

            uT_sb = u_pool.tile([P, KCH, C], dt.bfloat16)
            for pc in range(KCH):
                ps = ps_conv.tile([P, C], dt.float32, tag="conv", bufs=5)
                for cc in range(CC):
                    nc.tensor.matmul(ps[:], x_cols(cc, pc * P, P), wuT_sb[:, cc, :],
                                     start=(cc == 0), stop=(cc == CC - 1))
                if pc % 2 == 0:
                    nc.scalar.copy(uT_sb[:, pc, :], ps[:])
                else:
                    nc.vector.tensor_copy(uT_sb[:, pc, :], ps[:])
            wu_stack.close()

            conv_ps_stack.close()
            af_stack.close()
            ho_stack.close()
            hq_stack.close()

            # ---------------- Phase 3: attention ----------------
            at_stack = ExitStack()
            e_pool = at_stack.enter_context(tc.tile_pool(name="e_pool", bufs=3, side="left"))
            wT_pool = at_stack.enter_context(tc.tile_pool(name="wT_pool", bufs=3, side="left"))
            smax = at_stack.enter_context(tc.tile_pool(name="smax", bufs=2, side="left"))
            outp = at_stack.enter_context(tc.tile_pool(name="outp", bufs=2, side="left"))
            xqr = at_stack.enter_context(tc.tile_pool(name="xqr", bufs=3, side="left"))
            ps_sc = at_stack.enter_context(
                tc.tile_pool(name="ps_sc", bufs=2, space="PSUM", side="left"))
            ps_at = at_stack.enter_context(
                tc.tile_pool(name="ps_at", bufs=2, space="PSUM", side="left"))

            def scores_softmax(gi):
                """scores + online softmax for query chunk gi; returns the
                [key, query] weight tile written by the DMA XBAR transpose."""
                e_q = e_pool.tile([P, NQW, NKQ], dt.bfloat16, tag="e")
                mq = smax.tile([P, NQW], dt.float32, tag="mq")
                sq = smax.tile([P, NQW], dt.float32, tag="sq")
                bias_t = smax.tile([P, NQW], dt.float32, tag="bias")
                for w in range(NQW):
                    ps = ps_sc.tile([P, NKQ], dt.float32, tag="sc", bufs=2)
                    for half in range(2):
                        col0 = w * NKQ + half * 512
                        for cc in range(CC):
                            nc.tensor.matmul(
                                ps[:, half * 512:(half + 1) * 512],
                                q_t[cc][:, gi * P:(gi + 1) * P],
                                k_t[cc][:, col0:col0 + 512],
                                start=(cc == 0), stop=(cc == CC - 1))
                    nc.vector.reduce_max(out=mq[:, w:w + 1], in_=ps[:],
                                         axis=mybir.AxisListType.X)
                    nc.vector.tensor_scalar_mul(bias_t[:, w:w + 1], mq[:, w:w + 1],
                                                -SCALE)
                    nc.scalar.activation(
                        out=e_q[:, w, :], in_=ps[:],
                        func=mybir.ActivationFunctionType.Exp,
                        bias=bias_t[:, w:w + 1], scale=SCALE,
                        accum_out=sq[:, w:w + 1])
                # combine quarters: m = max_w mq ; alpha_w = exp(SCALE*(mq-m))/s
                m_t = smax.tile([P, 1], dt.float32, tag="m")
                nc.vector.reduce_max(out=m_t[:], in_=mq[:], axis=mybir.AxisListType.X)
                mb = smax.tile([P, 1], dt.float32, tag="mb")
                nc.vector.tensor_scalar_mul(mb[:], m_t[:], -SCALE)
                beta = smax.tile([P, NQW], dt.float32, tag="beta")
                nc.scalar.activation(out=beta[:], in_=mq[:],
                                     func=mybir.ActivationFunctionType.Exp,
                                     bias=mb[:], scale=SCALE)
                sb_t = smax.tile([P, NQW], dt.float32, tag="sbt")
                nc.vector.tensor_mul(sb_t[:], sq[:], beta[:])
                s_t = smax.tile([P, 1], dt.float32, tag="s")
                nc.vector.reduce_sum(out=s_t[:], in_=sb_t[:], axis=mybir.AxisListType.X)
                rs = smax.tile([P, 1], dt.float32, tag="rs")
                nc.vector.reciprocal(rs[:], s_t[:])
                alpha = smax.tile([P, NQW], dt.float32, tag="alpha")
                nc.vector.tensor_scalar_mul(alpha[:], beta[:], rs[:])
                for w in range(NQW):
                    nc.vector.tensor_scalar_mul(e_q[:, w, :], e_q[:, w, :],
                                                alpha[:, w:w + 1])
                # [query, key] -> [key, query] on the DMA XBAR:
                # wT[kp, kc, j] = e_q[j, kc*128 + kp]
                wT = wT_pool.tile([P, KCH, P], dt.bfloat16, tag="wT")
                nc.sync.dma_start(out=wT[:], in_=e_q[:], transpose=True)
                # prefetch the residual tile and fold the u bias into it,
                # off the attention critical path (gpsimd is slow but idle)
                xqr_sb = xqr.tile([P, C], dt.float32, tag="xqr")
                nc.sync.dma_start(out=xqr_sb[:],
                                  in_=xqT_ext[gi * P:(gi + 1) * P, :])
                nc.gpsimd.tensor_add(xqr_sb[:], xqr_sb[:], cb_row[:])
                return wT, xqr_sb

            def attn_out(gi, wT, xqr_sb):
                """outT[q, c] = sum_k wts[q, k] u[k, c]  (+ cb + x residual)."""
                ps = ps_at.tile([P, C], dt.float32, tag="at", bufs=2)
                for kc in range(KCH):
                    nc.tensor.matmul(ps[:], wT[:, kc, :], uT_sb[:, kc, :],
                                     start=(kc == 0), stop=(kc == KCH - 1))
                o_sb = outp.tile([P, C], dt.float32, tag="o")
                nc.vector.tensor_add(o_sb[:], ps[:], xqr_sb[:])
                nc.sync.dma_start(out=out_ext[gi * P:(gi + 1) * P, :], in_=o_sb[:])

            pend = {}
            for gi in range(QCH):
                pend[gi] = scores_softmax(gi)
                if gi >= 2:
                    attn_out(gi - 2, *pend.pop(gi - 2))
            for gi in (QCH - 2, QCH - 1):
                attn_out(gi, *pend.pop(gi))

            at_stack.close()
            top.close()

    # Force every activation onto the natural_log_exp_and_others table set so
    # the kernel never pays a mid-run ACT table swap (~2.7us each).
    import concourse.bacc as bacc_mod
    orig_tables = bacc_mod.get_activation_tables

    def one_set_tables(arch):
        t = dict(orig_tables(arch))
        return {name: (funcs if name == "natural_log_exp_and_others" else frozenset())
                for name, funcs in t.items()}

    bacc_mod.get_activation_tables = one_set_tables
    try:
        nc.compile()
    finally:
        bacc_mod.get_activation_tables = orig_tables
    return nc


def _get_nc(qk_mode, repeats=1):
    key = (qk_mode, repeats)
    if key not in _CACHE:
        _CACHE[key] = _build(qk_mode, repeats)
    return _CACHE[key]


def kernel(x, gn_weight, gn_bias, wq, bq, wk, bk, wv, bv, wo, bo):
    from concourse.bass_utils import run_bass_kernel_spmd

    nc = _get_nc(QK_MODE)

    x = np.asarray(x, dtype=np.float32)
    f32 = lambda a: np.ascontiguousarray(np.asarray(a, dtype=np.float32))

    wq64 = np.asarray(wq, dtype=np.float64)
    wk64 = np.asarray(wk, dtype=np.float64)
    wv64 = np.asarray(wv, dtype=np.float64)
    wo64 = np.asarray(wo, dtype=np.float64)
    wu64 = wo64 @ wv64                      # fused v + out conv
    bu = wo64 @ np.asarray(bv, np.float64) + np.asarray(bo, np.float64)

    wqT = f32(wq64.T)
    wkT = f32(wk64.T)
    wuT = f32(wu64.T)
    biases = f32(np.stack([np.asarray(bq, np.float64), bu], axis=1))  # [C, 2]
    gn_ab = f32(np.stack([gn_weight, gn_bias], axis=1))               # [C, 2]

    gsel = np.zeros((C, NUM_GROUPS), dtype=np.float32)
    gsel[np.arange(C), np.arange(C) // GSIZE] = 1.0 / GSIZE
    esel = np.zeros((NUM_GROUPS, C), dtype=np.float32)
    esel[np.arange(C) // GSIZE, np.arange(C)] = 1.0

    in_maps = []
    for core in range(8):
        b, half = core // 2, core % 2
        xb = x[b].reshape(C, N)
        xqb = f32(xb[:, half * NQ:(half + 1) * NQ])
        xob = f32(xb[:, (1 - half) * NQ:(2 - half) * NQ])
        in_maps.append({
            "xq": xqb, "xo": xob, "xqT": f32(xqb.T),
            "wqT": wqT, "wkT": wkT, "wuT": wuT,
            "biases": biases, "gn_ab": gn_ab, "gsel": gsel, "esel": esel,
        })

    import os
    trace = bool(os.environ.get("BASS_TRACE"))
    res = run_bass_kernel_spmd(nc, in_maps, core_ids=list(range(8)),
                               trace=trace)
    global _LAST
    _LAST = res

    out = np.empty((B, C, N), dtype=np.float32)
    for core in range(8):
        b, half = core // 2, core % 2
        out[b, :, half * NQ:(half + 1) * NQ] = res.results[core]["out"].T
    return out.reshape(B, C, H, W)


# revision 4
# speedup vs baseline: 1.1671x; 1.0360x over previous
"""AttnBlock (GroupNorm -> single-head attention over 64x64 pixels -> out conv
-> residual) on 8 Trainium2 NeuronCores.

Sharding: data parallel over batch (B=4) x 2-way split of the query-pixel axis
=> 8 cores, no collectives.  Each core receives its batch element's pixels as
two inputs: xq (its 2048 query columns) and xo (the other 2048 columns).  K and
the fused V·out-conv ("u") are computed over all 4096 pixels in the core-local
order [xq | xo].

Key structure (v2):
  * out-conv is fused into v on the host: wu = wo @ wv, so the attention
    matmul directly produces the final (pre-residual) output.
  * attention output is computed TRANSPOSED ([query, channel]) so each
    128-query chunk's weight matrix is an 8KB tile -> fine-grained pipeline.
  * the e-matrix [query, key] -> [key, query] transpose runs on the DMA XBAR
    (dma_start(transpose=True)), not the PE / ACT / DVE engines.
  * the k-conv bias is dropped: a per-key-channel bias shifts every score of
    a query row by the same amount -> softmax invariant.

All shapes hardcoded: B=4, C=512, H=W=64, N=4096, 32 groups.
"""

import numpy as np

B, C, H, W = 4, 512, 64, 64
N = H * W              # 4096 pixels
NQ = N // 2            # 2048 query pixels per core
NUM_GROUPS = 32
GSIZE = C // NUM_GROUPS  # 16 channels per group
EPS = 1e-6
SCALE = float(C) ** 0.5  # reference multiplies scores by sqrt(C)

P = 128                # partitions
CC = C // P            # 4 channel chunks
QCH = NQ // P          # 16 query chunks per core
NKQ = 1024             # k-columns per score quarter
NQW = N // NKQ         # 4 quarters per query chunk
KCH = N // P           # 32 key chunks

# "fp32r" = fast reduced-precision fp32 matmul for the q/k/scores path;
# "fp32" = full precision (4x slower on PE for the scores matmuls).
QK_MODE = "fp32r"

_CACHE = {}
_LAST = None


def _build(qk_mode, repeats=1):
    from contextlib import ExitStack

    import concourse.bacc as bacc
    import concourse.tile as tile
    from concourse import mybir
    from concourse.masks import make_identity

    dt = mybir.dt
    qk_dt = dt.float32r if qk_mode == "fp32r" else dt.float32

    nc = bacc.Bacc()
    xq_ext = nc.declare_dram_parameter("xq", [C, NQ], dt.float32, isOutput=False)
    xo_ext = nc.declare_dram_parameter("xo", [C, NQ], dt.float32, isOutput=False)
    xqT_ext = nc.declare_dram_parameter("xqT", [NQ, C], dt.float32, isOutput=False)
    wqT_ext = nc.declare_dram_parameter("wqT", [C, C], dt.float32, isOutput=False)
    wkT_ext = nc.declare_dram_parameter("wkT", [C, C], dt.float32, isOutput=False)
    wuT_ext = nc.declare_dram_parameter("wuT", [C, C], dt.float32, isOutput=False)
    biases_ext = nc.declare_dram_parameter("biases", [C, 2], dt.float32, isOutput=False)
    gn_ab_ext = nc.declare_dram_parameter("gn_ab", [C, 2], dt.float32, isOutput=False)
    gsel_ext = nc.declare_dram_parameter("gsel", [C, NUM_GROUPS], dt.float32, isOutput=False)
    esel_ext = nc.declare_dram_parameter("esel", [NUM_GROUPS, C], dt.float32, isOutput=False)
    out_ext = nc.declare_dram_parameter("out", [NQ, C], dt.float32, isOutput=True)

    with tile.TileContext(nc) as tc:
        # LEFT side: long-lived pools (whole kernel / attention phase).
        # RIGHT side: phase-scoped pools (GN scratch, conv weights, x).
        for _rep in range(repeats):
            top = ExitStack()
            const = top.enter_context(tc.tile_pool(name="const", bufs=1, side="left"))
            biases_sb = const.tile([P, CC, 2], dt.float32)  # [:, :, 0..1] = bq, bu
            nc.sync.dma_start(out=biases_sb[:], in_=biases_ext.rearrange("(c p) k -> p c k", p=P))
            k_pool = top.enter_context(tc.tile_pool(name="k_pool", bufs=1, side="left"))
            u_pool = top.enter_context(tc.tile_pool(name="u_pool", bufs=1, side="left"))
            q_pool = top.enter_context(tc.tile_pool(name="q_pool", bufs=1, side="left"))

            # ---------------- Phase 1: GroupNorm stats (folded into convs) ------
            # GroupNorm h = a*x + b is folded into the conv weights:
            #   W' = W @ diag(a),  bias' = W @ b + bias
            # so K/Q/U are computed directly from x and h never materializes.
            bx_sb = const.tile([P, CC, 2], dt.float32)  # folded conv biases q, u
            cb_row = const.tile([P, C], dt.float32)     # u bias broadcast per query row
            ident = const.tile([P, P], dt.float32)
            make_identity(nc, ident[:])

            hq_stack = ExitStack()
            hq_pool = hq_stack.enter_context(tc.tile_pool(name="hq_pool", bufs=1, side="right"))
            ho_stack = ExitStack()
            ho_pool = ho_stack.enter_context(tc.tile_pool(name="ho_pool", bufs=1, side="right"))
            af_stack = ExitStack()
            affine = af_stack.enter_context(tc.tile_pool(name="affine", bufs=1, side="right"))
            gn_stack = ExitStack()
            stat_pool = gn_stack.enter_context(tc.tile_pool(name="stat_pool", bufs=1, side="right"))
            small = gn_stack.enter_context(tc.tile_pool(name="small", bufs=1, side="right"))
            ps_small = gn_stack.enter_context(
                tc.tile_pool(name="ps_small", bufs=1, space="PSUM", side="right"))

            gsel_sb = small.tile([P, CC, NUM_GROUPS], dt.float32)
            nc.sync.dma_start(out=gsel_sb[:], in_=gsel_ext.rearrange("(c p) g -> p c g", p=P))
            esel_sb = small.tile([NUM_GROUPS, C], dt.float32)
            nc.sync.dma_start(out=esel_sb[:], in_=esel_ext[:])
            gn_ab_sb = small.tile([P, CC, 2], dt.float32)
            nc.sync.dma_start(out=gn_ab_sb[:], in_=gn_ab_ext.rearrange("(c p) k -> p c k", p=P))
            eps_sb = small.tile([NUM_GROUPS, 1], dt.float32)
            nc.vector.memset(eps_sb[:], EPS)

            xq_t, xo_t, ab_t = [], [], []
            for cc in range(CC):
                xqt = hq_pool.tile([P, NQ], qk_dt, name=f"hq_{cc}", tag=f"hq_{cc}")
                xot = ho_pool.tile([P, NQ], qk_dt, name=f"ho_{cc}", tag=f"ho_{cc}")
                for hcol in range(2):
                    cs = slice(hcol * NQ // 2, (hcol + 1) * NQ // 2)
                    nc.sync.dma_start(out=xqt[:, cs],
                                      in_=xq_ext[cc * P:(cc + 1) * P, cs].bitcast(qk_dt))
                    nc.sync.dma_start(out=xot[:, cs],
                                      in_=xo_ext[cc * P:(cc + 1) * P, cs].bitcast(qk_dt))
                xq_t.append(xqt)
                xo_t.append(xot)

            for cc in range(CC):
                xqf = xq_t[cc][:].bitcast(dt.float32)
                xof = xo_t[cc][:].bitcast(dt.float32)
                stats = stat_pool.tile([P, 8, 6], dt.float32, name=f"st_{cc}", tag="st", bufs=2)
                for j in range(4):
                    nc.vector.bn_stats(out=stats[:, j, :], in_=xqf[:, j * 512:(j + 1) * 512])
                for j in range(4):
                    nc.vector.bn_stats(out=stats[:, 4 + j, :], in_=xof[:, j * 512:(j + 1) * 512])
                mv = stat_pool.tile([P, 2], dt.float32, name=f"mv_{cc}", tag="mv", bufs=2)
                nc.vector.bn_aggr(out=mv[:], in_=stats[:])
                # mv[:,1] := var + mean^2  (per-channel second moment)
                sq = stat_pool.tile([P, 1], dt.float32, name=f"sq_{cc}", tag="sq", bufs=2)
                nc.vector.tensor_mul(sq[:], mv[:, 0:1], mv[:, 0:1])
                nc.vector.tensor_add(mv[:, 1:2], mv[:, 1:2], sq[:])

                # this chunk's 8 groups: [32, 2] = sum_c gsel[c,g] * mv[c,:]
                gps = ps_small.tile([NUM_GROUPS, 2], dt.float32, tag="gps", bufs=1)
                nc.tensor.matmul(gps[:], gsel_sb[:, cc, :], mv[:], start=True, stop=True)
                g_sb = small.tile([NUM_GROUPS, 2], dt.float32, name=f"g_{cc}", tag="g", bufs=2)
                nc.scalar.copy(g_sb[:], gps[:])
                # var_g = E[x^2] - mean^2 ; rstd = exp(-0.5*ln(var+eps))
                # (ln+exp live in one ACT table set; sqrt would force a table swap)
                gm2 = small.tile([NUM_GROUPS, 1], dt.float32, name=f"gm2_{cc}", tag="gm2", bufs=2)
                nc.vector.tensor_mul(gm2[:], g_sb[:, 0:1], g_sb[:, 0:1])
                grp = small.tile([NUM_GROUPS, 2], dt.float32, name=f"grp_{cc}", tag="grp", bufs=2)
                nc.vector.tensor_copy(grp[:, 0:1], g_sb[:, 0:1])
                varg = small.tile([NUM_GROUPS, 1], dt.float32, name=f"varg_{cc}", tag="varg", bufs=2)
                nc.vector.tensor_sub(varg[:], g_sb[:, 1:2], gm2[:])
                lng = small.tile([NUM_GROUPS, 1], dt.float32, name=f"lng_{cc}", tag="lng", bufs=2)
                nc.scalar.activation(lng[:], varg[:], mybir.ActivationFunctionType.Ln,
                                     bias=eps_sb[:], scale=1.0)
                nc.scalar.activation(grp[:, 1:2], lng[:], mybir.ActivationFunctionType.Exp,
                                     bias=0.0, scale=-0.5)

                # broadcast (mean, rstd) to this chunk's channels; GN affine fold:
                # a = gnw*rstd ; b = gnb - mean*a
                pcs = ps_small.tile([P, 2], dt.float32, tag="pcs", bufs=1)
                nc.tensor.matmul(pcs[:], esel_sb[:, cc * P:(cc + 1) * P], grp[:],
                                 start=True, stop=True)
                pc = small.tile([P, 2], dt.float32, name=f"pc_{cc}", tag="pc", bufs=2)
                nc.scalar.copy(pc[:], pcs[:])
                ab = affine.tile([P, 2], dt.float32, name=f"ab_{cc}", tag=f"ab_{cc}")
                nc.vector.tensor_mul(ab[:, 0:1], gn_ab_sb[:, cc, 0:1], pc[:, 1:2])
                t0 = small.tile([P, 1], dt.float32, name=f"t0_{cc}", tag="t0", bufs=2)
                nc.vector.tensor_mul(t0[:], pc[:, 0:1], ab[:, 0:1])
                nc.vector.tensor_sub(ab[:, 1:2], gn_ab_sb[:, cc, 1:2], t0[:])
                ab_t.append(ab)

            gn_stack.close()

            def x_cols(cc, col0, width):
                """x[cc][:, col0:col0+width] in the core-local order [xq | xo]."""
                if col0 < NQ:
                    return xq_t[cc][:, col0:col0 + width]
                return xo_t[cc][:, col0 - NQ:col0 - NQ + width]

            # ---------------- Phase 2: K / Q / U convs (from x directly) --------
            conv_ps_stack = ExitStack()
            ps_conv = conv_ps_stack.enter_context(
                tc.tile_pool(name="ps_conv", bufs=5, space="PSUM", side="right"))

            def fold_weight(wT_sb, bias_col):
                """bias' = sum_cc W_cc^T b_cc (+host bias), accumulated in
                PSUM; then scale W_cc in place (W'[c, o] = W[c, o] * a[c])."""
                if bias_col is not None:
                    for oc in range(CC):
                        bps = ps_conv.tile([P, 1], dt.float32, tag="bps", bufs=1)
                        for cc in range(CC):
                            nc.tensor.matmul(bps[:],
                                             wT_sb[:, cc, oc * P:(oc + 1) * P].bitcast(dt.float32),
                                             ab_t[cc][:, 1:2],
                                             start=(cc == 0), stop=(cc == CC - 1))
                        nc.vector.tensor_add(bx_sb[:, oc, bias_col:bias_col + 1],
                                             bps[:],
                                             biases_sb[:, oc, bias_col:bias_col + 1])
                for cc in range(CC):
                    nc.vector.tensor_scalar_mul(wT_sb[:, cc, :],
                                                wT_sb[:, cc, :].bitcast(dt.float32),
                                                ab_t[cc][:, 0:1])

            # --- K conv (no bias: per-key-channel bias is softmax-invariant) ---
            wk_stack = ExitStack()
            wk_pool = wk_stack.enter_context(tc.tile_pool(name="wk_pool", bufs=1, side="right"))
            wkT_sb = wk_pool.tile([P, CC, C], qk_dt)
            for cc in range(CC):
                nc.sync.dma_start(out=wkT_sb[:, cc, :],
                                  in_=wkT_ext[cc * P:(cc + 1) * P, :].bitcast(qk_dt))
            fold_weight(wkT_sb, None)

            k_t = [k_pool.tile([P, N], qk_dt, name=f"k_{oc}", tag=f"k_{oc}") for oc in range(CC)]
            for oc in range(CC):
                for ncol in range(N // 512):
                    ps = ps_conv.tile([P, 512], dt.float32, tag="conv", bufs=5)
                    for cc in range(CC):
                        nc.tensor.matmul(ps[:], wkT_sb[:, cc, oc * P:(oc + 1) * P],
                                         x_cols(cc, ncol * 512, 512),
                                         start=(cc == 0), stop=(cc == CC - 1))
                    dst = k_t[oc][:, ncol * 512:(ncol + 1) * 512]
                    if ncol % 2 == 0:
                        nc.vector.tensor_copy(dst, ps[:])
                    else:
                        nc.scalar.copy(dst, ps[:])
            wk_stack.close()

            # --- Q conv (bias kept: shifts scores per-key -> not invariant) ---
            wq_stack = ExitStack()
            wq_pool = wq_stack.enter_context(tc.tile_pool(name="wq_pool", bufs=1, side="right"))
            wqT_sb = wq_pool.tile([P, CC, C], qk_dt)
            for cc in range(CC):
                nc.sync.dma_start(out=wqT_sb[:, cc, :],
                                  in_=wqT_ext[cc * P:(cc + 1) * P, :].bitcast(qk_dt))
            fold_weight(wqT_sb, 0)

            q_t = [q_pool.tile([P, NQ], qk_dt, name=f"q_{oc}", tag=f"q_{oc}") for oc in range(CC)]
            for oc in range(CC):
                for ncol in range(NQ // 512):
                    ps = ps_conv.tile([P, 512], dt.float32, tag="conv", bufs=5)
                    for cc in range(CC):
                        nc.tensor.matmul(ps[:], wqT_sb[:, cc, oc * P:(oc + 1) * P],
                                         xq_t[cc][:, ncol * 512:(ncol + 1) * 512],
                                         start=(cc == 0), stop=(cc == CC - 1))
                    dst = q_t[oc][:, ncol * 512:(ncol + 1) * 512]
                    if ncol % 2 == 0:
                        nc.vector.tensor_scalar(
                            out=dst, in0=ps[:],
                            scalar1=bx_sb[:, oc, 0:1], scalar2=None,
                            op0=mybir.AluOpType.add)
                    else:
                        nc.scalar.activation(
                            out=dst, in_=ps[:],
                            func=mybir.ActivationFunctionType.Identity,
                            bias=bx_sb[:, oc, 0:1], scale=1.0)
            wq_stack.close()

            # --- U conv: u = (wo@wv)' @ x, [pixel, channel] layout, bf16.
            # The u bias (wu@b + wo@bv + bo) is NOT applied here: softmax
            # weights sum to 1, so it is added once at the end via cb_row.
            wu_stack = ExitStack()
            wu_pool = wu_stack.enter_context(tc.tile_pool(name="wu_pool", bufs=1, side="right"))
            wuT_sb = wu_pool.tile([P, CC, C], qk_dt)
            for cc in range(CC):
                nc.sync.dma_start(out=wuT_sb[:, cc, :],
                                  in_=wuT_ext[cc * P:(cc + 1) * P, :].bitcast(qk_dt))
            fold_weight(wuT_sb, 1)

            # --- cb_row: broadcast the folded u bias to all 128 query rows.
            # bx_sb[:, oc, 1] is [128, 1] per chunk; PE-transpose each to
            # [1, 128] then broadcast-matmul with a ones column.
            cb_stack = ExitStack()
            cb_pool = cb_stack.enter_context(tc.tile_pool(name="cb_pool", bufs=1, side="right"))
            ps_cb = cb_stack.enter_context(
                tc.tile_pool(name="ps_cb", bufs=1, space="PSUM", side="right"))
            tp1 = ps_cb.tile([1, CC, P], dt.float32, tag="tp1")
            for oc in range(CC):
                nc.tensor.transpose(tp1[:, oc, :], bx_sb[:, oc, 1:2], ident[:])
            cb1 = cb_pool.tile([1, CC, P], dt.float32)
            nc.scalar.copy(cb1[:], tp1[:])
            ones1 = cb_pool.tile([1, P], dt.float32)
            nc.vector.memset(ones1[:], 1.0)
            cb_ps = ps_cb.tile([P, C], dt.float32, tag="cbps")
            for oc in range(CC):
                nc.tensor.matmul(cb_ps[:, oc * P:(oc + 1) * P], ones1[:],
                                 cb1[:, oc, :], start=True, stop=True)
            nc.scalar.copy(cb_row[:], cb_ps[:])
            cb_stack.close()

            uT_sb = u_pool.tile([P, KCH, C], dt.bfloat16)
            for pc in range(KCH):
                ps = ps_conv.tile([P, C], dt.float32, tag="conv", bufs=5)
                for cc in range(CC):
                    nc.tensor.matmul(ps[:], x_cols(cc, pc * P, P), wuT_sb[:, cc, :],
                                     start=(cc == 0), stop=(cc == CC - 1))
                if pc % 2 == 0:
                    nc.scalar.copy(uT_sb[:, pc, :], ps[:])
                else:
                    nc.vector.tensor_copy(uT_sb[:, pc, :], ps[:])
            wu_stack.close()

            conv_ps_stack.close()
            af_stack.close()
            ho_stack.close()
            hq_stack.close()

            # ---------------- Phase 3: attention ----------------
            at_stack = ExitStack()
            e_pool = at_stack.enter_context(tc.tile_pool(name="e_pool", bufs=3, side="left"))
            wT_pool = at_stack.enter_context(tc.tile_pool(name="wT_pool", bufs=4, side="left"))
            smax = at_stack.enter_context(tc.tile_pool(name="smax", bufs=2, side="left"))
            outp = at_stack.enter_context(tc.tile_pool(name="outp", bufs=2, side="left"))
            xqr = at_stack.enter_context(tc.tile_pool(name="xqr", bufs=4, side="left"))
            ps_sc = at_stack.enter_context(
                tc.tile_pool(name="ps_sc", bufs=2, space="PSUM", side="left"))
            ps_at = at_stack.enter_context(
                tc.tile_pool(name="ps_at", bufs=2, space="PSUM", side="left"))

            def scores_softmax(gi):
                """scores + online softmax for query chunk gi; returns the
                [key, query] weight tile written by the DMA XBAR transpose."""
                e_q = e_pool.tile([P, NQW, NKQ], dt.bfloat16, tag="e")
                mq = smax.tile([P, NQW], dt.float32, tag="mq")
                sq = smax.tile([P, NQW], dt.float32, tag="sq")
                bias_t = smax.tile([P, NQW], dt.float32, tag="bias")
                for w in range(NQW):
                    ps = ps_sc.tile([P, NKQ], dt.float32, tag="sc", bufs=2)
                    for half in range(2):
                        col0 = w * NKQ + half * 512
                        for cc in range(CC):
                            nc.tensor.matmul(
                                ps[:, half * 512:(half + 1) * 512],
                                q_t[cc][:, gi * P:(gi + 1) * P],
                                k_t[cc][:, col0:col0 + 512],
                                start=(cc == 0), stop=(cc == CC - 1))
                    nc.vector.reduce_max(out=mq[:, w:w + 1], in_=ps[:],
                                         axis=mybir.AxisListType.X)
                    nc.vector.tensor_scalar_mul(bias_t[:, w:w + 1], mq[:, w:w + 1],
                                                -SCALE)
                    nc.scalar.activation(
                        out=e_q[:, w, :], in_=ps[:],
                        func=mybir.ActivationFunctionType.Exp,
                        bias=bias_t[:, w:w + 1], scale=SCALE,
                        accum_out=sq[:, w:w + 1])
                # combine quarters: m = max_w mq ; beta_w = exp(SCALE*(mq-m)).
                # e is rescaled by beta only; the uniform per-query 1/s lands
                # on the attention OUTPUT, whose partition axis is queries.
                m_t = smax.tile([P, 1], dt.float32, tag="m")
                nc.vector.reduce_max(out=m_t[:], in_=mq[:], axis=mybir.AxisListType.X)
                mb = smax.tile([P, 1], dt.float32, tag="mb")
                nc.vector.tensor_scalar_mul(mb[:], m_t[:], -SCALE)
                beta = smax.tile([P, NQW], dt.float32, tag="beta")
                nc.scalar.activation(out=beta[:], in_=mq[:],
                                     func=mybir.ActivationFunctionType.Exp,
                                     bias=mb[:], scale=SCALE)
                for w in range(NQW):
                    nc.vector.tensor_scalar_mul(e_q[:, w, :], e_q[:, w, :],
                                                beta[:, w:w + 1])
                sb_t = smax.tile([P, NQW], dt.float32, tag="sbt")
                nc.vector.tensor_mul(sb_t[:], sq[:], beta[:])
                s_t = smax.tile([P, 1], dt.float32, tag="s")
                nc.vector.reduce_sum(out=s_t[:], in_=sb_t[:], axis=mybir.AxisListType.X)
                rs = xqr.tile([P, 1], dt.float32, tag="rs")
                nc.vector.reciprocal(rs[:], s_t[:])
                # [query, key] -> [key, query] on the DMA XBAR:
                # wT[kp, kc, j] = e_q[j, kc*128 + kp]
                wT = wT_pool.tile([P, KCH, P], dt.bfloat16, tag="wT")
                nc.sync.dma_start(out=wT[:], in_=e_q[:], transpose=True)
                # prefetch the residual tile and fold the u bias into it,
                # off the attention critical path (gpsimd is slow but idle)
                xqr_sb = xqr.tile([P, C], dt.float32, tag="xqr")
                nc.sync.dma_start(out=xqr_sb[:],
                                  in_=xqT_ext[gi * P:(gi + 1) * P, :])
                nc.gpsimd.tensor_add(xqr_sb[:], xqr_sb[:], cb_row[:])
                return wT, xqr_sb, rs

            def attn_out(gi, wT, xqr_sb, rs):
                """outT[q, c] = (sum_k e[q, k] u[k, c]) / s[q] + cb + x."""
                ps = ps_at.tile([P, C], dt.float32, tag="at", bufs=2)
                for kc in range(KCH):
                    nc.tensor.matmul(ps[:], wT[:, kc, :], uT_sb[:, kc, :],
                                     start=(kc == 0), stop=(kc == KCH - 1))
                o_sb = outp.tile([P, C], dt.float32, tag="o")
                nc.vector.tensor_scalar_mul(o_sb[:], ps[:], rs[:])
                nc.vector.tensor_add(o_sb[:], o_sb[:], xqr_sb[:])
                nc.sync.dma_start(out=out_ext[gi * P:(gi + 1) * P, :], in_=o_sb[:])

            LAG = 3
            pend = {}
            for gi in range(QCH):
                pend[gi] = scores_softmax(gi)
                if gi >= LAG:
                    attn_out(gi - LAG, *pend.pop(gi - LAG))
            for gi in range(QCH - LAG, QCH):
                attn_out(gi, *pend.pop(gi))

            at_stack.close()
            top.close()

    # Force every activation onto the natural_log_exp_and_others table set so
    # the kernel never pays a mid-run ACT table swap (~2.7us each).
    import concourse.bacc as bacc_mod
    orig_tables = bacc_mod.get_activation_tables

    def one_set_tables(arch):
        t = dict(orig_tables(arch))
        return {name: (funcs if name == "natural_log_exp_and_others" else frozenset())
                for name, funcs in t.items()}

    bacc_mod.get_activation_tables = one_set_tables
    try:
        nc.compile()
    finally:
        bacc_mod.get_activation_tables = orig_tables
    return nc


def _get_nc(qk_mode, repeats=1):
    key = (qk_mode, repeats)
    if key not in _CACHE:
        _CACHE[key] = _build(qk_mode, repeats)
    return _CACHE[key]


def kernel(x, gn_weight, gn_bias, wq, bq, wk, bk, wv, bv, wo, bo):
    from concourse.bass_utils import run_bass_kernel_spmd

    nc = _get_nc(QK_MODE)

    x = np.asarray(x, dtype=np.float32)
    f32 = lambda a: np.ascontiguousarray(np.asarray(a, dtype=np.float32))

    wq64 = np.asarray(wq, dtype=np.float64)
    wk64 = np.asarray(wk, dtype=np.float64)
    wv64 = np.asarray(wv, dtype=np.float64)
    wo64 = np.asarray(wo, dtype=np.float64)
    wu64 = wo64 @ wv64                      # fused v + out conv
    bu = wo64 @ np.asarray(bv, np.float64) + np.asarray(bo, np.float64)

    wqT = f32(wq64.T)
    wkT = f32(wk64.T)
    wuT = f32(wu64.T)
    biases = f32(np.stack([np.asarray(bq, np.float64), bu], axis=1))  # [C, 2]
    gn_ab = f32(np.stack([gn_weight, gn_bias], axis=1))               # [C, 2]

    gsel = np.zeros((C, NUM_GROUPS), dtype=np.float32)
    gsel[np.arange(C), np.arange(C) // GSIZE] = 1.0 / GSIZE
    esel = np.zeros((NUM_GROUPS, C), dtype=np.float32)
    esel[np.arange(C) // GSIZE, np.arange(C)] = 1.0

    in_maps = []
    for core in range(8):
        b, half = core // 2, core % 2
        xb = x[b].reshape(C, N)
        xqb = f32(xb[:, half * NQ:(half + 1) * NQ])
        xob = f32(xb[:, (1 - half) * NQ:(2 - half) * NQ])
        in_maps.append({
            "xq": xqb, "xo": xob, "xqT": f32(xqb.T),
            "wqT": wqT, "wkT": wkT, "wuT": wuT,
            "biases": biases, "gn_ab": gn_ab, "gsel": gsel, "esel": esel,
        })

    import os
    trace = bool(os.environ.get("BASS_TRACE"))
    res = run_bass_kernel_spmd(nc, in_maps, core_ids=list(range(8)),
                               trace=trace)
    global _LAST
    _LAST = res

    out = np.empty((B, C, N), dtype=np.float32)
    for core in range(8):
        b, half = core // 2, core % 2
        out[b, :, half * NQ:(half + 1) * NQ] = res.results[core]["out"].T
    return out.reshape(B, C, H, W)


# revision 5
# speedup vs baseline: 1.2118x; 1.0384x over previous
"""AttnBlock (GroupNorm -> single-head attention over 64x64 pixels -> out conv
-> residual) on 8 Trainium2 NeuronCores.

Sharding: data parallel over batch (B=4) x 2-way split of the query-pixel axis
=> 8 cores, no collectives.  Each core receives its batch element's pixels as
two inputs: xq (its 2048 query columns) and xo (the other 2048 columns).  K and
the fused V·out-conv ("u") are computed over all 4096 pixels in the core-local
order [xq | xo].

Key structure (v2):
  * out-conv is fused into v on the host: wu = wo @ wv, so the attention
    matmul directly produces the final (pre-residual) output.
  * attention output is computed TRANSPOSED ([query, channel]) so each
    128-query chunk's weight matrix is an 8KB tile -> fine-grained pipeline.
  * the e-matrix [query, key] -> [key, query] transpose runs on the DMA XBAR
    (dma_start(transpose=True)), not the PE / ACT / DVE engines.
  * the k-conv bias is dropped: a per-key-channel bias shifts every score of
    a query row by the same amount -> softmax invariant.

All shapes hardcoded: B=4, C=512, H=W=64, N=4096, 32 groups.
"""

import numpy as np

B, C, H, W = 4, 512, 64, 64
N = H * W              # 4096 pixels
NQ = N // 2            # 2048 query pixels per core
NUM_GROUPS = 32
GSIZE = C // NUM_GROUPS  # 16 channels per group
EPS = 1e-6
SCALE = float(C) ** 0.5  # reference multiplies scores by sqrt(C)

P = 128                # partitions
CC = C // P            # 4 channel chunks
QCH = NQ // P          # 16 query chunks per core
NKQ = 1024             # k-columns per score quarter
NQW = N // NKQ         # 4 quarters per query chunk
KCH = N // P           # 32 key chunks

# "fp32r" = fast reduced-precision fp32 matmul for the q/k/scores path;
# "fp32" = full precision (4x slower on PE for the scores matmuls).
QK_MODE = "fp32r"

_CACHE = {}
_LAST = None


def _build(qk_mode, repeats=1):
    from contextlib import ExitStack

    import concourse.bacc as bacc
    import concourse.tile as tile
    from concourse import mybir
    from concourse.masks import make_identity

    dt = mybir.dt
    qk_dt = dt.float32r if qk_mode == "fp32r" else dt.float32

    nc = bacc.Bacc()
    xq_ext = nc.declare_dram_parameter("xq", [C, NQ], dt.float32, isOutput=False)
    xo_ext = nc.declare_dram_parameter("xo", [C, NQ], dt.float32, isOutput=False)
    xqT_ext = nc.declare_dram_parameter("xqT", [NQ, C], dt.float32, isOutput=False)
    wqT_ext = nc.declare_dram_parameter("wqT", [C, C], dt.float32, isOutput=False)
    wkT_ext = nc.declare_dram_parameter("wkT", [C, C], dt.float32, isOutput=False)
    wuT_ext = nc.declare_dram_parameter("wuT", [C, C], dt.float32, isOutput=False)
    biases_ext = nc.declare_dram_parameter("biases", [C, 2], dt.float32, isOutput=False)
    gn_ab_ext = nc.declare_dram_parameter("gn_ab", [C, 2], dt.float32, isOutput=False)
    gsel_ext = nc.declare_dram_parameter("gsel", [C, NUM_GROUPS], dt.float32, isOutput=False)
    esel_ext = nc.declare_dram_parameter("esel", [NUM_GROUPS, C], dt.float32, isOutput=False)
    out_ext = nc.declare_dram_parameter("out", [NQ, C], dt.float32, isOutput=True)

    with tile.TileContext(nc) as tc:
        # LEFT side: long-lived pools (whole kernel / attention phase).
        # RIGHT side: phase-scoped pools (GN scratch, conv weights, x).
        for _rep in range(repeats):
            top = ExitStack()
            const = top.enter_context(tc.tile_pool(name="const", bufs=1, side="left"))
            biases_sb = const.tile([P, CC, 2], dt.float32)  # [:, :, 0..1] = bq, bu
            nc.sync.dma_start(out=biases_sb[:], in_=biases_ext.rearrange("(c p) k -> p c k", p=P))
            k_pool = top.enter_context(tc.tile_pool(name="k_pool", bufs=1, side="left"))
            q_pool = top.enter_context(tc.tile_pool(name="q_pool", bufs=1, side="left"))
            # u weight in a long-lived left pool: its DMA has no WAR on the
            # other conv weights, so it lands during the k conv.  u_pool
            # itself (32KB) is created just before the u conv.
            wu_pool = top.enter_context(tc.tile_pool(name="wu_pool", bufs=1, side="left"))

            # ---------------- Phase 1: GroupNorm stats (folded into convs) ------
            # GroupNorm h = a*x + b is folded into the conv weights:
            #   W' = W @ diag(a),  bias' = W @ b + bias
            # so K/Q/U are computed directly from x and h never materializes.
            bx_sb = const.tile([P, CC, 2], dt.float32)  # folded conv biases q, u
            cb_row = const.tile([P, C], dt.float32)     # u bias broadcast per query row
            ident = const.tile([P, P], dt.float32)
            make_identity(nc, ident[:])

            hq_stack = ExitStack()
            hq_pool = hq_stack.enter_context(tc.tile_pool(name="hq_pool", bufs=1, side="right"))
            ho_stack = ExitStack()
            ho_pool = ho_stack.enter_context(tc.tile_pool(name="ho_pool", bufs=1, side="right"))
            af_stack = ExitStack()
            affine = af_stack.enter_context(tc.tile_pool(name="affine", bufs=1, side="right"))
            # weight pools below the GN scratch in the right-side stack:
            # close order is GN scratch -> wk (after k conv) -> wq (after q)
            wq_stack = ExitStack()
            wq_pool = wq_stack.enter_context(tc.tile_pool(name="wq_pool", bufs=1, side="right"))
            wk_stack = ExitStack()
            wk_pool = wk_stack.enter_context(tc.tile_pool(name="wk_pool", bufs=1, side="right"))
            gn_stack = ExitStack()
            stat_pool = gn_stack.enter_context(tc.tile_pool(name="stat_pool", bufs=1, side="right"))
            small = gn_stack.enter_context(tc.tile_pool(name="small", bufs=1, side="right"))
            ps_small = gn_stack.enter_context(
                tc.tile_pool(name="ps_small", bufs=1, space="PSUM", side="right"))

            gsel_sb = small.tile([P, CC, NUM_GROUPS], dt.float32)
            nc.sync.dma_start(out=gsel_sb[:], in_=gsel_ext.rearrange("(c p) g -> p c g", p=P))
            esel_sb = small.tile([NUM_GROUPS, C], dt.float32)
            nc.sync.dma_start(out=esel_sb[:], in_=esel_ext[:])
            gn_ab_sb = small.tile([P, CC, 2], dt.float32)
            nc.sync.dma_start(out=gn_ab_sb[:], in_=gn_ab_ext.rearrange("(c p) k -> p c k", p=P))
            eps_sb = small.tile([NUM_GROUPS, 1], dt.float32)
            nc.vector.memset(eps_sb[:], EPS)

            xq_t, xo_t, ab_t = [], [], []
            for cc in range(CC):
                xqt = hq_pool.tile([P, NQ], qk_dt, name=f"hq_{cc}", tag=f"hq_{cc}")
                xot = ho_pool.tile([P, NQ], qk_dt, name=f"ho_{cc}", tag=f"ho_{cc}")
                for hcol in range(2):
                    cs = slice(hcol * NQ // 2, (hcol + 1) * NQ // 2)
                    nc.sync.dma_start(out=xqt[:, cs],
                                      in_=xq_ext[cc * P:(cc + 1) * P, cs].bitcast(qk_dt))
                    nc.sync.dma_start(out=xot[:, cs],
                                      in_=xo_ext[cc * P:(cc + 1) * P, cs].bitcast(qk_dt))
                xq_t.append(xqt)
                xo_t.append(xot)

            # all three weight DMAs up front — no WAR on each other's space,
            # they land during the x load / k conv
            wkT_sb = wk_pool.tile([P, CC, C], qk_dt)
            for cc in range(CC):
                nc.sync.dma_start(out=wkT_sb[:, cc, :],
                                  in_=wkT_ext[cc * P:(cc + 1) * P, :].bitcast(qk_dt))
            wqT_sb = wq_pool.tile([P, CC, C], qk_dt)
            for cc in range(CC):
                nc.sync.dma_start(out=wqT_sb[:, cc, :],
                                  in_=wqT_ext[cc * P:(cc + 1) * P, :].bitcast(qk_dt))
            wuT_sb = wu_pool.tile([P, CC, C], qk_dt)
            for cc in range(CC):
                nc.sync.dma_start(out=wuT_sb[:, cc, :],
                                  in_=wuT_ext[cc * P:(cc + 1) * P, :].bitcast(qk_dt))

            for cc in range(CC):
                xqf = xq_t[cc][:].bitcast(dt.float32)
                xof = xo_t[cc][:].bitcast(dt.float32)
                stats = stat_pool.tile([P, 8, 6], dt.float32, name=f"st_{cc}", tag="st", bufs=2)
                for j in range(4):
                    nc.vector.bn_stats(out=stats[:, j, :], in_=xqf[:, j * 512:(j + 1) * 512])
                for j in range(4):
                    nc.vector.bn_stats(out=stats[:, 4 + j, :], in_=xof[:, j * 512:(j + 1) * 512])
                mv = stat_pool.tile([P, 2], dt.float32, name=f"mv_{cc}", tag="mv", bufs=2)
                nc.vector.bn_aggr(out=mv[:], in_=stats[:])
                # mv[:,1] := var + mean^2  (per-channel second moment)
                sq = stat_pool.tile([P, 1], dt.float32, name=f"sq_{cc}", tag="sq", bufs=2)
                nc.vector.tensor_mul(sq[:], mv[:, 0:1], mv[:, 0:1])
                nc.vector.tensor_add(mv[:, 1:2], mv[:, 1:2], sq[:])

                # this chunk's 8 groups: [32, 2] = sum_c gsel[c,g] * mv[c,:]
                gps = ps_small.tile([NUM_GROUPS, 2], dt.float32, tag="gps", bufs=1)
                nc.tensor.matmul(gps[:], gsel_sb[:, cc, :], mv[:], start=True, stop=True)
                g_sb = small.tile([NUM_GROUPS, 2], dt.float32, name=f"g_{cc}", tag="g", bufs=2)
                nc.scalar.copy(g_sb[:], gps[:])
                # var_g = E[x^2] - mean^2 ; rstd = exp(-0.5*ln(var+eps))
                # (ln+exp live in one ACT table set; sqrt would force a table swap)
                gm2 = small.tile([NUM_GROUPS, 1], dt.float32, name=f"gm2_{cc}", tag="gm2", bufs=2)
                nc.vector.tensor_mul(gm2[:], g_sb[:, 0:1], g_sb[:, 0:1])
                grp = small.tile([NUM_GROUPS, 2], dt.float32, name=f"grp_{cc}", tag="grp", bufs=2)
                nc.vector.tensor_copy(grp[:, 0:1], g_sb[:, 0:1])
                varg = small.tile([NUM_GROUPS, 1], dt.float32, name=f"varg_{cc}", tag="varg", bufs=2)
                nc.vector.tensor_sub(varg[:], g_sb[:, 1:2], gm2[:])
                lng = small.tile([NUM_GROUPS, 1], dt.float32, name=f"lng_{cc}", tag="lng", bufs=2)
                nc.scalar.activation(lng[:], varg[:], mybir.ActivationFunctionType.Ln,
                                     bias=eps_sb[:], scale=1.0)
                nc.scalar.activation(grp[:, 1:2], lng[:], mybir.ActivationFunctionType.Exp,
                                     bias=0.0, scale=-0.5)

                # broadcast (mean, rstd) to this chunk's channels; GN affine fold:
                # a = gnw*rstd ; b = gnb - mean*a
                pcs = ps_small.tile([P, 2], dt.float32, tag="pcs", bufs=1)
                nc.tensor.matmul(pcs[:], esel_sb[:, cc * P:(cc + 1) * P], grp[:],
                                 start=True, stop=True)
                pc = small.tile([P, 2], dt.float32, name=f"pc_{cc}", tag="pc", bufs=2)
                nc.scalar.copy(pc[:], pcs[:])
                ab = affine.tile([P, 2], dt.float32, name=f"ab_{cc}", tag=f"ab_{cc}")
                nc.vector.tensor_mul(ab[:, 0:1], gn_ab_sb[:, cc, 0:1], pc[:, 1:2])
                t0 = small.tile([P, 1], dt.float32, name=f"t0_{cc}", tag="t0", bufs=2)
                nc.vector.tensor_mul(t0[:], pc[:, 0:1], ab[:, 0:1])
                nc.vector.tensor_sub(ab[:, 1:2], gn_ab_sb[:, cc, 1:2], t0[:])
                ab_t.append(ab)

            gn_stack.close()

            def x_cols(cc, col0, width):
                """x[cc][:, col0:col0+width] in the core-local order [xq | xo]."""
                if col0 < NQ:
                    return xq_t[cc][:, col0:col0 + width]
                return xo_t[cc][:, col0 - NQ:col0 - NQ + width]

            # ---------------- Phase 2: K / Q / U convs (from x directly) --------
            conv_ps_stack = ExitStack()
            ps_conv = conv_ps_stack.enter_context(
                tc.tile_pool(name="ps_conv", bufs=5, space="PSUM", side="right"))

            def fold_weight(wT_sb, bias_col):
                """bias' = sum_cc W_cc^T b_cc (+host bias), accumulated in
                PSUM; then scale W_cc in place (W'[c, o] = W[c, o] * a[c])."""
                if bias_col is not None:
                    for oc in range(CC):
                        bps = ps_conv.tile([P, 1], dt.float32, tag="bps", bufs=1)
                        for cc in range(CC):
                            nc.tensor.matmul(bps[:],
                                             wT_sb[:, cc, oc * P:(oc + 1) * P].bitcast(dt.float32),
                                             ab_t[cc][:, 1:2],
                                             start=(cc == 0), stop=(cc == CC - 1))
                        nc.vector.tensor_add(bx_sb[:, oc, bias_col:bias_col + 1],
                                             bps[:],
                                             biases_sb[:, oc, bias_col:bias_col + 1])
                for cc in range(CC):
                    nc.vector.tensor_scalar_mul(wT_sb[:, cc, :],
                                                wT_sb[:, cc, :].bitcast(dt.float32),
                                                ab_t[cc][:, 0:1])

            # --- K conv (no bias: per-key-channel bias is softmax-invariant) ---
            fold_weight(wkT_sb, None)
            fold_weight(wqT_sb, 0)

            k_t = [k_pool.tile([P, N], qk_dt, name=f"k_{oc}", tag=f"k_{oc}") for oc in range(CC)]
            for oc in range(CC):
                for ncol in range(N // 512):
                    ps = ps_conv.tile([P, 512], dt.float32, tag="conv", bufs=5)
                    for cc in range(CC):
                        nc.tensor.matmul(ps[:], wkT_sb[:, cc, oc * P:(oc + 1) * P],
                                         x_cols(cc, ncol * 512, 512),
                                         start=(cc == 0), stop=(cc == CC - 1))
                    dst = k_t[oc][:, ncol * 512:(ncol + 1) * 512]
                    if ncol % 2 == 0:
                        nc.vector.tensor_copy(dst, ps[:])
                    else:
                        nc.scalar.copy(dst, ps[:])
            wk_stack.close()

            # --- Q conv (bias kept: shifts scores per-key -> not invariant) ---
            q_t = [q_pool.tile([P, NQ], qk_dt, name=f"q_{oc}", tag=f"q_{oc}") for oc in range(CC)]
            for oc in range(CC):
                for ncol in range(NQ // 512):
                    ps = ps_conv.tile([P, 512], dt.float32, tag="conv", bufs=5)
                    for cc in range(CC):
                        nc.tensor.matmul(ps[:], wqT_sb[:, cc, oc * P:(oc + 1) * P],
                                         xq_t[cc][:, ncol * 512:(ncol + 1) * 512],
                                         start=(cc == 0), stop=(cc == CC - 1))
                    dst = q_t[oc][:, ncol * 512:(ncol + 1) * 512]
                    if ncol % 2 == 0:
                        nc.vector.tensor_scalar(
                            out=dst, in0=ps[:],
                            scalar1=bx_sb[:, oc, 0:1], scalar2=None,
                            op0=mybir.AluOpType.add)
                    else:
                        nc.scalar.activation(
                            out=dst, in_=ps[:],
                            func=mybir.ActivationFunctionType.Identity,
                            bias=bx_sb[:, oc, 0:1], scale=1.0)
            wq_stack.close()

            # --- U conv: u = (wo@wv)' @ x, [pixel, channel] layout, bf16.
            # The u bias (wu@b + wo@bv + bo) is NOT applied here: softmax
            # weights sum to 1, so it is added once at the end via cb_row.
            fold_weight(wuT_sb, 1)

            # --- cb_row: broadcast the folded u bias to all 128 query rows.
            # bx_sb[:, oc, 1] is [128, 1] per chunk; PE-transpose each to
            # [1, 128] then broadcast-matmul with a ones column.
            cb_stack = ExitStack()
            cb_pool = cb_stack.enter_context(tc.tile_pool(name="cb_pool", bufs=1, side="right"))
            ps_cb = cb_stack.enter_context(
                tc.tile_pool(name="ps_cb", bufs=1, space="PSUM", side="right"))
            tp1 = ps_cb.tile([1, CC, P], dt.float32, tag="tp1")
            for oc in range(CC):
                nc.tensor.transpose(tp1[:, oc, :], bx_sb[:, oc, 1:2], ident[:])
            cb1 = cb_pool.tile([1, CC, P], dt.float32)
            nc.scalar.copy(cb1[:], tp1[:])
            ones1 = cb_pool.tile([1, P], dt.float32)
            nc.vector.memset(ones1[:], 1.0)
            cb_ps = ps_cb.tile([P, C], dt.float32, tag="cbps")
            for oc in range(CC):
                nc.tensor.matmul(cb_ps[:, oc * P:(oc + 1) * P], ones1[:],
                                 cb1[:, oc, :], start=True, stop=True)
            nc.scalar.copy(cb_row[:], cb_ps[:])
            cb_stack.close()

            u_pool = top.enter_context(tc.tile_pool(name="u_pool", bufs=1, side="left"))
            uT_sb = u_pool.tile([P, KCH, C], dt.bfloat16)
            for pc in range(KCH):
                ps = ps_conv.tile([P, C], dt.float32, tag="conv", bufs=5)
                for cc in range(CC):
                    nc.tensor.matmul(ps[:], x_cols(cc, pc * P, P), wuT_sb[:, cc, :],
                                     start=(cc == 0), stop=(cc == CC - 1))
                if pc % 2 == 0:
                    nc.scalar.copy(uT_sb[:, pc, :], ps[:])
                else:
                    nc.vector.tensor_copy(uT_sb[:, pc, :], ps[:])

            conv_ps_stack.close()
            af_stack.close()
            ho_stack.close()
            hq_stack.close()

            # ---------------- Phase 3: attention ----------------
            at_stack = ExitStack()
            e_pool = at_stack.enter_context(tc.tile_pool(name="e_pool", bufs=3, side="left"))
            wT_pool = at_stack.enter_context(tc.tile_pool(name="wT_pool", bufs=4, side="left"))
            smax = at_stack.enter_context(tc.tile_pool(name="smax", bufs=2, side="left"))
            outp = at_stack.enter_context(tc.tile_pool(name="outp", bufs=2, side="left"))
            xqr = at_stack.enter_context(tc.tile_pool(name="xqr", bufs=4, side="left"))
            ps_sc = at_stack.enter_context(
                tc.tile_pool(name="ps_sc", bufs=2, space="PSUM", side="left"))
            ps_at = at_stack.enter_context(
                tc.tile_pool(name="ps_at", bufs=2, space="PSUM", side="left"))

            def scores_softmax(gi):
                """scores + online softmax for query chunk gi; returns the
                [key, query] weight tile written by the DMA XBAR transpose."""
                e_q = e_pool.tile([P, NQW, NKQ], dt.bfloat16, tag="e")
                mq = smax.tile([P, NQW], dt.float32, tag="mq")
                sq = smax.tile([P, NQW], dt.float32, tag="sq")
                bias_t = smax.tile([P, NQW], dt.float32, tag="bias")
                for w in range(NQW):
                    ps = ps_sc.tile([P, NKQ], dt.float32, tag="sc", bufs=2)
                    for half in range(2):
                        col0 = w * NKQ + half * 512
                        for cc in range(CC):
                            nc.tensor.matmul(
                                ps[:, half * 512:(half + 1) * 512],
                                q_t[cc][:, gi * P:(gi + 1) * P],
                                k_t[cc][:, col0:col0 + 512],
                                start=(cc == 0), stop=(cc == CC - 1))
                    nc.vector.reduce_max(out=mq[:, w:w + 1], in_=ps[:],
                                         axis=mybir.AxisListType.X)
                    nc.vector.tensor_scalar_mul(bias_t[:, w:w + 1], mq[:, w:w + 1],
                                                -SCALE)
                    nc.scalar.activation(
                        out=e_q[:, w, :], in_=ps[:],
                        func=mybir.ActivationFunctionType.Exp,
                        bias=bias_t[:, w:w + 1], scale=SCALE,
                        accum_out=sq[:, w:w + 1])
                # combine quarters: m = max_w mq ; beta_w = exp(SCALE*(mq-m)).
                # e is rescaled by beta only; the uniform per-query 1/s lands
                # on the attention OUTPUT, whose partition axis is queries.
                m_t = smax.tile([P, 1], dt.float32, tag="m")
                nc.vector.reduce_max(out=m_t[:], in_=mq[:], axis=mybir.AxisListType.X)
                mb = smax.tile([P, 1], dt.float32, tag="mb")
                nc.vector.tensor_scalar_mul(mb[:], m_t[:], -SCALE)
                beta = smax.tile([P, NQW], dt.float32, tag="beta")
                nc.scalar.activation(out=beta[:], in_=mq[:],
                                     func=mybir.ActivationFunctionType.Exp,
                                     bias=mb[:], scale=SCALE)
                for w in range(NQW):
                    nc.vector.tensor_scalar_mul(e_q[:, w, :], e_q[:, w, :],
                                                beta[:, w:w + 1])
                sb_t = smax.tile([P, NQW], dt.float32, tag="sbt")
                nc.vector.tensor_mul(sb_t[:], sq[:], beta[:])
                s_t = smax.tile([P, 1], dt.float32, tag="s")
                nc.vector.reduce_sum(out=s_t[:], in_=sb_t[:], axis=mybir.AxisListType.X)
                rs = xqr.tile([P, 1], dt.float32, tag="rs")
                nc.vector.reciprocal(rs[:], s_t[:])
                # [query, key] -> [key, query] on the DMA XBAR:
                # wT[kp, kc, j] = e_q[j, kc*128 + kp]
                wT = wT_pool.tile([P, KCH, P], dt.bfloat16, tag="wT")
                nc.sync.dma_start(out=wT[:], in_=e_q[:], transpose=True)
                # prefetch the residual tile and fold the u bias into it,
                # off the attention critical path (gpsimd is slow but idle)
                xqr_sb = xqr.tile([P, C], dt.float32, tag="xqr")
                nc.sync.dma_start(out=xqr_sb[:],
                                  in_=xqT_ext[gi * P:(gi + 1) * P, :])
                nc.gpsimd.tensor_add(xqr_sb[:], xqr_sb[:], cb_row[:])
                return wT, xqr_sb, rs

            def attn_out(gi, wT, xqr_sb, rs):
                """outT[q, c] = (sum_k e[q, k] u[k, c]) / s[q] + cb + x."""
                ps = ps_at.tile([P, C], dt.float32, tag="at", bufs=2)
                for kc in range(KCH):
                    nc.tensor.matmul(ps[:], wT[:, kc, :], uT_sb[:, kc, :],
                                     start=(kc == 0), stop=(kc == KCH - 1))
                o_sb = outp.tile([P, C], dt.float32, tag="o")
                nc.vector.tensor_scalar_mul(o_sb[:], ps[:], rs[:])
                nc.vector.tensor_add(o_sb[:], o_sb[:], xqr_sb[:])
                nc.sync.dma_start(out=out_ext[gi * P:(gi + 1) * P, :], in_=o_sb[:])

            LAG = 3
            pend = {}
            for gi in range(QCH):
                pend[gi] = scores_softmax(gi)
                if gi >= LAG:
                    attn_out(gi - LAG, *pend.pop(gi - LAG))
            for gi in range(QCH - LAG, QCH):
                attn_out(gi, *pend.pop(gi))

            at_stack.close()
            top.close()

    # Force every activation onto the natural_log_exp_and_others table set so
    # the kernel never pays a mid-run ACT table swap (~2.7us each).
    import concourse.bacc as bacc_mod
    orig_tables = bacc_mod.get_activation_tables

    def one_set_tables(arch):
        t = dict(orig_tables(arch))
        return {name: (funcs if name == "natural_log_exp_and_others" else frozenset())
                for name, funcs in t.items()}

    bacc_mod.get_activation_tables = one_set_tables
    try:
        nc.compile()
    finally:
        bacc_mod.get_activation_tables = orig_tables
    return nc


def _get_nc(qk_mode, repeats=1):
    key = (qk_mode, repeats)
    if key not in _CACHE:
        _CACHE[key] = _build(qk_mode, repeats)
    return _CACHE[key]


def kernel(x, gn_weight, gn_bias, wq, bq, wk, bk, wv, bv, wo, bo):
    from concourse.bass_utils import run_bass_kernel_spmd

    nc = _get_nc(QK_MODE)

    x = np.asarray(x, dtype=np.float32)
    f32 = lambda a: np.ascontiguousarray(np.asarray(a, dtype=np.float32))

    wq64 = np.asarray(wq, dtype=np.float64)
    wk64 = np.asarray(wk, dtype=np.float64)
    wv64 = np.asarray(wv, dtype=np.float64)
    wo64 = np.asarray(wo, dtype=np.float64)
    wu64 = wo64 @ wv64                      # fused v + out conv
    bu = wo64 @ np.asarray(bv, np.float64) + np.asarray(bo, np.float64)

    wqT = f32(wq64.T)
    wkT = f32(wk64.T)
    wuT = f32(wu64.T)
    biases = f32(np.stack([np.asarray(bq, np.float64), bu], axis=1))  # [C, 2]
    gn_ab = f32(np.stack([gn_weight, gn_bias], axis=1))               # [C, 2]

    gsel = np.zeros((C, NUM_GROUPS), dtype=np.float32)
    gsel[np.arange(C), np.arange(C) // GSIZE] = 1.0 / GSIZE
    esel = np.zeros((NUM_GROUPS, C), dtype=np.float32)
    esel[np.arange(C) // GSIZE, np.arange(C)] = 1.0

    in_maps = []
    for core in range(8):
        b, half = core // 2, core % 2
        xb = x[b].reshape(C, N)
        xqb = f32(xb[:, half * NQ:(half + 1) * NQ])
        xob = f32(xb[:, (1 - half) * NQ:(2 - half) * NQ])
        in_maps.append({
            "xq": xqb, "xo": xob, "xqT": f32(xqb.T),
            "wqT": wqT, "wkT": wkT, "wuT": wuT,
            "biases": biases, "gn_ab": gn_ab, "gsel": gsel, "esel": esel,
        })

    import os
    trace = bool(os.environ.get("BASS_TRACE"))
    res = run_bass_kernel_spmd(nc, in_maps, core_ids=list(range(8)),
                               trace=trace)
    global _LAST
    _LAST = res

    out = np.empty((B, C, N), dtype=np.float32)
    for core in range(8):
        b, half = core // 2, core % 2
        out[b, :, half * NQ:(half + 1) * NQ] = res.results[core]["out"].T
    return out.reshape(B, C, H, W)


# revision 6
# speedup vs baseline: 1.2160x; 1.0034x over previous
"""AttnBlock (GroupNorm -> single-head attention over 64x64 pixels -> out conv
-> residual) on 8 Trainium2 NeuronCores.

Sharding: data parallel over batch (B=4) x 2-way split of the query-pixel axis
=> 8 cores, no collectives.  Each core receives its batch element's pixels as
two inputs: xq (its 2048 query columns) and xo (the other 2048 columns).  K and
the fused V·out-conv ("u") are computed over all 4096 pixels in the core-local
order [xq | xo].

Key structure (v2):
  * out-conv is fused into v on the host: wu = wo @ wv, so the attention
    matmul directly produces the final (pre-residual) output.
  * attention output is computed TRANSPOSED ([query, channel]) so each
    128-query chunk's weight matrix is an 8KB tile -> fine-grained pipeline.
  * the e-matrix [query, key] -> [key, query] transpose runs on the DMA XBAR
    (dma_start(transpose=True)), not the PE / ACT / DVE engines.
  * the k-conv bias is dropped: a per-key-channel bias shifts every score of
    a query row by the same amount -> softmax invariant.

All shapes hardcoded: B=4, C=512, H=W=64, N=4096, 32 groups.
"""

import numpy as np

B, C, H, W = 4, 512, 64, 64
N = H * W              # 4096 pixels
NQ = N // 2            # 2048 query pixels per core
NUM_GROUPS = 32
GSIZE = C // NUM_GROUPS  # 16 channels per group
EPS = 1e-6
SCALE = float(C) ** 0.5  # reference multiplies scores by sqrt(C)

P = 128                # partitions
CC = C // P            # 4 channel chunks
QCH = NQ // P          # 16 query chunks per core
NKQ = 1024             # k-columns per score quarter
NQW = N // NKQ         # 4 quarters per query chunk
KCH = N // P           # 32 key chunks

# "fp32r" = fast reduced-precision fp32 matmul for the q/k/scores path;
# "fp32" = full precision (4x slower on PE for the scores matmuls).
QK_MODE = "fp32r"

_CACHE = {}
_LAST = None


def _build(qk_mode, repeats=1):
    from contextlib import ExitStack

    import concourse.bacc as bacc
    import concourse.tile as tile
    from concourse import mybir
    from concourse.masks import make_identity

    dt = mybir.dt
    qk_dt = dt.float32r if qk_mode == "fp32r" else dt.float32

    nc = bacc.Bacc()
    xq_ext = nc.declare_dram_parameter("xq", [C, NQ], dt.float32, isOutput=False)
    xo_ext = nc.declare_dram_parameter("xo", [C, NQ], dt.float32, isOutput=False)
    xqT_ext = nc.declare_dram_parameter("xqT", [NQ, C], dt.float32, isOutput=False)
    wqT_ext = nc.declare_dram_parameter("wqT", [C, C], dt.float32, isOutput=False)
    wkT_ext = nc.declare_dram_parameter("wkT", [C, C], dt.float32, isOutput=False)
    wuT_ext = nc.declare_dram_parameter("wuT", [C, C], dt.float32, isOutput=False)
    biases_ext = nc.declare_dram_parameter("biases", [C, 2], dt.float32, isOutput=False)
    gn_ab_ext = nc.declare_dram_parameter("gn_ab", [C, 2], dt.float32, isOutput=False)
    gsel_ext = nc.declare_dram_parameter("gsel", [C, NUM_GROUPS], dt.float32, isOutput=False)
    esel_ext = nc.declare_dram_parameter("esel", [NUM_GROUPS, C], dt.float32, isOutput=False)
    out_ext = nc.declare_dram_parameter("out", [NQ, C], dt.float32, isOutput=True)

    with tile.TileContext(nc) as tc:
        # LEFT side: long-lived pools (whole kernel / attention phase).
        # RIGHT side: phase-scoped pools (GN scratch, conv weights, x).
        for _rep in range(repeats):
            top = ExitStack()
            const = top.enter_context(tc.tile_pool(name="const", bufs=1, side="left"))
            biases_sb = const.tile([P, CC, 2], dt.float32)  # [:, :, 0..1] = bq, bu
            nc.sync.dma_start(out=biases_sb[:], in_=biases_ext.rearrange("(c p) k -> p c k", p=P))
            k_pool = top.enter_context(tc.tile_pool(name="k_pool", bufs=1, side="left"))
            q_pool = top.enter_context(tc.tile_pool(name="q_pool", bufs=1, side="left"))
            # u weight in a long-lived left pool: its DMA has no WAR on the
            # other conv weights, so it lands during the k conv.  u_pool
            # itself (32KB) is created just before the u conv.
            wu_pool = top.enter_context(tc.tile_pool(name="wu_pool", bufs=1, side="left"))

            # ---------------- Phase 1: GroupNorm stats (folded into convs) ------
            # GroupNorm h = a*x + b is folded into the conv weights:
            #   W' = W @ diag(a),  bias' = W @ b + bias
            # so K/Q/U are computed directly from x and h never materializes.
            bx_sb = const.tile([P, CC, 2], dt.float32)  # folded conv biases q, u
            cb_row = const.tile([P, C], dt.float32)     # u bias broadcast per query row
            ident = const.tile([P, P], dt.float32)
            make_identity(nc, ident[:])

            hq_stack = ExitStack()
            hq_pool = hq_stack.enter_context(tc.tile_pool(name="hq_pool", bufs=1, side="right"))
            ho_stack = ExitStack()
            ho_pool = ho_stack.enter_context(tc.tile_pool(name="ho_pool", bufs=1, side="right"))
            af_stack = ExitStack()
            affine = af_stack.enter_context(tc.tile_pool(name="affine", bufs=1, side="right"))
            # weight pools below the GN scratch in the right-side stack:
            # close order is GN scratch -> wk (after k conv) -> wq (after q)
            wq_stack = ExitStack()
            wq_pool = wq_stack.enter_context(tc.tile_pool(name="wq_pool", bufs=1, side="right"))
            wk_stack = ExitStack()
            wk_pool = wk_stack.enter_context(tc.tile_pool(name="wk_pool", bufs=1, side="right"))
            gn_stack = ExitStack()
            stat_pool = gn_stack.enter_context(tc.tile_pool(name="stat_pool", bufs=1, side="right"))
            small = gn_stack.enter_context(tc.tile_pool(name="small", bufs=1, side="right"))
            ps_small = gn_stack.enter_context(
                tc.tile_pool(name="ps_small", bufs=1, space="PSUM", side="right"))

            gsel_sb = small.tile([P, CC, NUM_GROUPS], dt.float32)
            nc.sync.dma_start(out=gsel_sb[:], in_=gsel_ext.rearrange("(c p) g -> p c g", p=P))
            esel_sb = small.tile([NUM_GROUPS, C], dt.float32)
            nc.sync.dma_start(out=esel_sb[:], in_=esel_ext[:])
            gn_ab_sb = small.tile([P, CC, 2], dt.float32)
            nc.sync.dma_start(out=gn_ab_sb[:], in_=gn_ab_ext.rearrange("(c p) k -> p c k", p=P))
            eps_sb = small.tile([NUM_GROUPS, 1], dt.float32)
            nc.vector.memset(eps_sb[:], EPS)

            xq_t, xo_t, ab_t = [], [], []
            for cc in range(CC):
                xqt = hq_pool.tile([P, NQ], qk_dt, name=f"hq_{cc}", tag=f"hq_{cc}")
                xot = ho_pool.tile([P, NQ], qk_dt, name=f"ho_{cc}", tag=f"ho_{cc}")
                for hcol in range(2):
                    cs = slice(hcol * NQ // 2, (hcol + 1) * NQ // 2)
                    nc.sync.dma_start(out=xqt[:, cs],
                                      in_=xq_ext[cc * P:(cc + 1) * P, cs].bitcast(qk_dt))
                    nc.sync.dma_start(out=xot[:, cs],
                                      in_=xo_ext[cc * P:(cc + 1) * P, cs].bitcast(qk_dt))
                xq_t.append(xqt)
                xo_t.append(xot)

            # all three weight DMAs up front — no WAR on each other's space,
            # they land during the x load / k conv
            wkT_sb = wk_pool.tile([P, CC, C], qk_dt)
            for cc in range(CC):
                nc.sync.dma_start(out=wkT_sb[:, cc, :],
                                  in_=wkT_ext[cc * P:(cc + 1) * P, :].bitcast(qk_dt))
            wqT_sb = wq_pool.tile([P, CC, C], qk_dt)
            for cc in range(CC):
                nc.sync.dma_start(out=wqT_sb[:, cc, :],
                                  in_=wqT_ext[cc * P:(cc + 1) * P, :].bitcast(qk_dt))
            wuT_sb = wu_pool.tile([P, CC, C], qk_dt)
            for cc in range(CC):
                nc.sync.dma_start(out=wuT_sb[:, cc, :],
                                  in_=wuT_ext[cc * P:(cc + 1) * P, :].bitcast(qk_dt))

            for cc in range(CC):
                xqf = xq_t[cc][:].bitcast(dt.float32)
                xof = xo_t[cc][:].bitcast(dt.float32)
                stats = stat_pool.tile([P, 8, 6], dt.float32, name=f"st_{cc}", tag="st", bufs=2)
                for j in range(4):
                    nc.vector.bn_stats(out=stats[:, j, :], in_=xqf[:, j * 512:(j + 1) * 512])
                for j in range(4):
                    nc.vector.bn_stats(out=stats[:, 4 + j, :], in_=xof[:, j * 512:(j + 1) * 512])
                mv = stat_pool.tile([P, 2], dt.float32, name=f"mv_{cc}", tag="mv", bufs=2)
                nc.vector.bn_aggr(out=mv[:], in_=stats[:])
                # mv[:,1] := var + mean^2  (per-channel second moment)
                sq = stat_pool.tile([P, 1], dt.float32, name=f"sq_{cc}", tag="sq", bufs=2)
                nc.vector.tensor_mul(sq[:], mv[:, 0:1], mv[:, 0:1])
                nc.vector.tensor_add(mv[:, 1:2], mv[:, 1:2], sq[:])

                # this chunk's 8 groups: [32, 2] = sum_c gsel[c,g] * mv[c,:]
                gps = ps_small.tile([NUM_GROUPS, 2], dt.float32, tag="gps", bufs=1)
                nc.tensor.matmul(gps[:], gsel_sb[:, cc, :], mv[:], start=True, stop=True)
                g_sb = small.tile([NUM_GROUPS, 2], dt.float32, name=f"g_{cc}", tag="g", bufs=2)
                nc.scalar.copy(g_sb[:], gps[:])
                # var_g = E[x^2] - mean^2 ; rstd = exp(-0.5*ln(var+eps))
                # (ln+exp live in one ACT table set; sqrt would force a table swap)
                gm2 = small.tile([NUM_GROUPS, 1], dt.float32, name=f"gm2_{cc}", tag="gm2", bufs=2)
                nc.vector.tensor_mul(gm2[:], g_sb[:, 0:1], g_sb[:, 0:1])
                grp = small.tile([NUM_GROUPS, 2], dt.float32, name=f"grp_{cc}", tag="grp", bufs=2)
                nc.vector.tensor_copy(grp[:, 0:1], g_sb[:, 0:1])
                varg = small.tile([NUM_GROUPS, 1], dt.float32, name=f"varg_{cc}", tag="varg", bufs=2)
                nc.vector.tensor_sub(varg[:], g_sb[:, 1:2], gm2[:])
                lng = small.tile([NUM_GROUPS, 1], dt.float32, name=f"lng_{cc}", tag="lng", bufs=2)
                nc.scalar.activation(lng[:], varg[:], mybir.ActivationFunctionType.Ln,
                                     bias=eps_sb[:], scale=1.0)
                nc.scalar.activation(grp[:, 1:2], lng[:], mybir.ActivationFunctionType.Exp,
                                     bias=0.0, scale=-0.5)

                # broadcast (mean, rstd) to this chunk's channels; GN affine fold:
                # a = gnw*rstd ; b = gnb - mean*a
                pcs = ps_small.tile([P, 2], dt.float32, tag="pcs", bufs=1)
                nc.tensor.matmul(pcs[:], esel_sb[:, cc * P:(cc + 1) * P], grp[:],
                                 start=True, stop=True)
                pc = small.tile([P, 2], dt.float32, name=f"pc_{cc}", tag="pc", bufs=2)
                nc.scalar.copy(pc[:], pcs[:])
                ab = affine.tile([P, 2], dt.float32, name=f"ab_{cc}", tag=f"ab_{cc}")
                nc.vector.tensor_mul(ab[:, 0:1], gn_ab_sb[:, cc, 0:1], pc[:, 1:2])
                t0 = small.tile([P, 1], dt.float32, name=f"t0_{cc}", tag="t0", bufs=2)
                nc.vector.tensor_mul(t0[:], pc[:, 0:1], ab[:, 0:1])
                nc.vector.tensor_sub(ab[:, 1:2], gn_ab_sb[:, cc, 1:2], t0[:])
                ab_t.append(ab)

            gn_stack.close()

            def x_cols(cc, col0, width):
                """x[cc][:, col0:col0+width] in the core-local order [xq | xo]."""
                if col0 < NQ:
                    return xq_t[cc][:, col0:col0 + width]
                return xo_t[cc][:, col0 - NQ:col0 - NQ + width]

            # ---------------- Phase 2: K / Q / U convs (from x directly) --------
            conv_ps_stack = ExitStack()
            ps_conv = conv_ps_stack.enter_context(
                tc.tile_pool(name="ps_conv", bufs=5, space="PSUM", side="right"))

            def fold_bias(wT_sb, bias_col):
                """bias' = sum_cc W_cc^T b_cc (+host bias), accumulated in
                PSUM.  Must run on the UNSCALED weights."""
                for oc in range(CC):
                    bps = ps_conv.tile([P, 1], dt.float32, tag="bps", bufs=1)
                    for cc in range(CC):
                        nc.tensor.matmul(bps[:],
                                         wT_sb[:, cc, oc * P:(oc + 1) * P].bitcast(dt.float32),
                                         ab_t[cc][:, 1:2],
                                         start=(cc == 0), stop=(cc == CC - 1))
                    nc.vector.tensor_add(bx_sb[:, oc, bias_col:bias_col + 1],
                                         bps[:],
                                         biases_sb[:, oc, bias_col:bias_col + 1])

            def fold_scale(wT_sb):
                """Scale W_cc in place (W'[c, o] = W[c, o] * a[c])."""
                for cc in range(CC):
                    nc.vector.tensor_scalar_mul(wT_sb[:, cc, :],
                                                wT_sb[:, cc, :].bitcast(dt.float32),
                                                ab_t[cc][:, 0:1])

            # --- K conv (no bias: per-key-channel bias is softmax-invariant) ---
            fold_scale(wkT_sb)

            k_t = [k_pool.tile([P, N], qk_dt, name=f"k_{oc}", tag=f"k_{oc}") for oc in range(CC)]
            for oc in range(CC):
                for ncol in range(N // 512):
                    ps = ps_conv.tile([P, 512], dt.float32, tag="conv", bufs=5)
                    for cc in range(CC):
                        nc.tensor.matmul(ps[:], wkT_sb[:, cc, oc * P:(oc + 1) * P],
                                         x_cols(cc, ncol * 512, 512),
                                         start=(cc == 0), stop=(cc == CC - 1))
                    dst = k_t[oc][:, ncol * 512:(ncol + 1) * 512]
                    if ncol % 2 == 0:
                        nc.vector.tensor_copy(dst, ps[:])
                    else:
                        nc.scalar.copy(dst, ps[:])
            wk_stack.close()

            # --- Q conv (bias kept: shifts scores per-key -> not invariant) ---
            # fold-bias matmuls sit AFTER the k conv in PE order: they are
            # gated on the same group stats, and emitting them first would
            # delay the first k-conv matmul by ~4us.
            fold_bias(wqT_sb, 0)
            fold_scale(wqT_sb)
            q_t = [q_pool.tile([P, NQ], qk_dt, name=f"q_{oc}", tag=f"q_{oc}") for oc in range(CC)]
            for oc in range(CC):
                for ncol in range(NQ // 512):
                    ps = ps_conv.tile([P, 512], dt.float32, tag="conv", bufs=5)
                    for cc in range(CC):
                        nc.tensor.matmul(ps[:], wqT_sb[:, cc, oc * P:(oc + 1) * P],
                                         xq_t[cc][:, ncol * 512:(ncol + 1) * 512],
                                         start=(cc == 0), stop=(cc == CC - 1))
                    dst = q_t[oc][:, ncol * 512:(ncol + 1) * 512]
                    if ncol % 2 == 0:
                        nc.vector.tensor_scalar(
                            out=dst, in0=ps[:],
                            scalar1=bx_sb[:, oc, 0:1], scalar2=None,
                            op0=mybir.AluOpType.add)
                    else:
                        nc.scalar.activation(
                            out=dst, in_=ps[:],
                            func=mybir.ActivationFunctionType.Identity,
                            bias=bx_sb[:, oc, 0:1], scale=1.0)
            wq_stack.close()

            # --- U conv: u = (wo@wv)' @ x, [pixel, channel] layout, bf16.
            # The u bias (wu@b + wo@bv + bo) is NOT applied here: softmax
            # weights sum to 1, so it is added once at the end via cb_row.
            fold_bias(wuT_sb, 1)
            fold_scale(wuT_sb)

            # --- cb_row: broadcast the folded u bias to all 128 query rows.
            # bx_sb[:, oc, 1] is [128, 1] per chunk; PE-transpose each to
            # [1, 128] then broadcast-matmul with a ones column.
            cb_stack = ExitStack()
            cb_pool = cb_stack.enter_context(tc.tile_pool(name="cb_pool", bufs=1, side="right"))
            ps_cb = cb_stack.enter_context(
                tc.tile_pool(name="ps_cb", bufs=1, space="PSUM", side="right"))
            tp1 = ps_cb.tile([1, CC, P], dt.float32, tag="tp1")
            for oc in range(CC):
                nc.tensor.transpose(tp1[:, oc, :], bx_sb[:, oc, 1:2], ident[:])
            cb1 = cb_pool.tile([1, CC, P], dt.float32)
            nc.scalar.copy(cb1[:], tp1[:])
            ones1 = cb_pool.tile([1, P], dt.float32)
            nc.vector.memset(ones1[:], 1.0)
            cb_ps = ps_cb.tile([P, C], dt.float32, tag="cbps")
            for oc in range(CC):
                nc.tensor.matmul(cb_ps[:, oc * P:(oc + 1) * P], ones1[:],
                                 cb1[:, oc, :], start=True, stop=True)
            nc.scalar.copy(cb_row[:], cb_ps[:])
            cb_stack.close()

            u_pool = top.enter_context(tc.tile_pool(name="u_pool", bufs=1, side="left"))
            uT_sb = u_pool.tile([P, KCH, C], dt.bfloat16)
            for pc in range(KCH):
                ps = ps_conv.tile([P, C], dt.float32, tag="conv", bufs=5)
                for cc in range(CC):
                    nc.tensor.matmul(ps[:], x_cols(cc, pc * P, P), wuT_sb[:, cc, :],
                                     start=(cc == 0), stop=(cc == CC - 1))
                if pc % 2 == 0:
                    nc.scalar.copy(uT_sb[:, pc, :], ps[:])
                else:
                    nc.vector.tensor_copy(uT_sb[:, pc, :], ps[:])

            conv_ps_stack.close()
            af_stack.close()
            ho_stack.close()
            hq_stack.close()

            # ---------------- Phase 3: attention ----------------
            at_stack = ExitStack()
            e_pool = at_stack.enter_context(tc.tile_pool(name="e_pool", bufs=3, side="left"))
            wT_pool = at_stack.enter_context(tc.tile_pool(name="wT_pool", bufs=4, side="left"))
            smax = at_stack.enter_context(tc.tile_pool(name="smax", bufs=2, side="left"))
            outp = at_stack.enter_context(tc.tile_pool(name="outp", bufs=2, side="left"))
            xqr = at_stack.enter_context(tc.tile_pool(name="xqr", bufs=4, side="left"))
            ps_sc = at_stack.enter_context(
                tc.tile_pool(name="ps_sc", bufs=3, space="PSUM", side="left"))
            ps_at = at_stack.enter_context(
                tc.tile_pool(name="ps_at", bufs=2, space="PSUM", side="left"))

            def scores_softmax(gi):
                """scores + online softmax for query chunk gi; returns the
                [key, query] weight tile written by the DMA XBAR transpose."""
                e_q = e_pool.tile([P, NQW, NKQ], dt.bfloat16, tag="e")
                mq = smax.tile([P, NQW], dt.float32, tag="mq")
                sq = smax.tile([P, NQW], dt.float32, tag="sq")
                bias_t = smax.tile([P, NQW], dt.float32, tag="bias")
                for w in range(NQW):
                    ps = ps_sc.tile([P, NKQ], dt.float32, tag="sc", bufs=3)
                    for half in range(2):
                        col0 = w * NKQ + half * 512
                        for cc in range(CC):
                            nc.tensor.matmul(
                                ps[:, half * 512:(half + 1) * 512],
                                q_t[cc][:, gi * P:(gi + 1) * P],
                                k_t[cc][:, col0:col0 + 512],
                                start=(cc == 0), stop=(cc == CC - 1))
                    nc.vector.reduce_max(out=mq[:, w:w + 1], in_=ps[:],
                                         axis=mybir.AxisListType.X)
                    nc.vector.tensor_scalar_mul(bias_t[:, w:w + 1], mq[:, w:w + 1],
                                                -SCALE)
                    nc.scalar.activation(
                        out=e_q[:, w, :], in_=ps[:],
                        func=mybir.ActivationFunctionType.Exp,
                        bias=bias_t[:, w:w + 1], scale=SCALE,
                        accum_out=sq[:, w:w + 1])
                # combine quarters: m = max_w mq ; beta_w = exp(SCALE*(mq-m)).
                # e is rescaled by beta only; the uniform per-query 1/s lands
                # on the attention OUTPUT, whose partition axis is queries.
                m_t = smax.tile([P, 1], dt.float32, tag="m")
                nc.vector.reduce_max(out=m_t[:], in_=mq[:], axis=mybir.AxisListType.X)
                mb = smax.tile([P, 1], dt.float32, tag="mb")
                nc.vector.tensor_scalar_mul(mb[:], m_t[:], -SCALE)
                beta = smax.tile([P, NQW], dt.float32, tag="beta")
                nc.scalar.activation(out=beta[:], in_=mq[:],
                                     func=mybir.ActivationFunctionType.Exp,
                                     bias=mb[:], scale=SCALE)
                for w in range(NQW):
                    nc.vector.tensor_scalar_mul(e_q[:, w, :], e_q[:, w, :],
                                                beta[:, w:w + 1])
                sb_t = smax.tile([P, NQW], dt.float32, tag="sbt")
                nc.vector.tensor_mul(sb_t[:], sq[:], beta[:])
                s_t = smax.tile([P, 1], dt.float32, tag="s")
                nc.vector.reduce_sum(out=s_t[:], in_=sb_t[:], axis=mybir.AxisListType.X)
                rs = xqr.tile([P, 1], dt.float32, tag="rs")
                nc.vector.reciprocal(rs[:], s_t[:])
                # [query, key] -> [key, query] on the DMA XBAR:
                # wT[kp, kc, j] = e_q[j, kc*128 + kp]
                wT = wT_pool.tile([P, KCH, P], dt.bfloat16, tag="wT")
                nc.sync.dma_start(out=wT[:], in_=e_q[:], transpose=True)
                # prefetch the residual tile and fold the u bias into it,
                # off the attention critical path (gpsimd is slow but idle)
                xqr_sb = xqr.tile([P, C], dt.float32, tag="xqr")
                nc.sync.dma_start(out=xqr_sb[:],
                                  in_=xqT_ext[gi * P:(gi + 1) * P, :])
                nc.gpsimd.tensor_add(xqr_sb[:], xqr_sb[:], cb_row[:])
                return wT, xqr_sb, rs

            def attn_out(gi, wT, xqr_sb, rs):
                """outT[q, c] = (sum_k e[q, k] u[k, c]) / s[q] + cb + x."""
                ps = ps_at.tile([P, C], dt.float32, tag="at", bufs=2)
                for kc in range(KCH):
                    nc.tensor.matmul(ps[:], wT[:, kc, :], uT_sb[:, kc, :],
                                     start=(kc == 0), stop=(kc == KCH - 1))
                o_sb = outp.tile([P, C], dt.float32, tag="o")
                nc.vector.tensor_scalar_mul(o_sb[:], ps[:], rs[:])
                nc.vector.tensor_add(o_sb[:], o_sb[:], xqr_sb[:])
                nc.sync.dma_start(out=out_ext[gi * P:(gi + 1) * P, :], in_=o_sb[:])

            LAG = 3
            pend = {}
            for gi in range(QCH):
                pend[gi] = scores_softmax(gi)
                if gi >= LAG:
                    attn_out(gi - LAG, *pend.pop(gi - LAG))
            for gi in range(QCH - LAG, QCH):
                attn_out(gi, *pend.pop(gi))

            at_stack.close()
            top.close()

    # Force every activation onto the natural_log_exp_and_others table set so
    # the kernel never pays a mid-run ACT table swap (~2.7us each).
    import concourse.bacc as bacc_mod
    orig_tables = bacc_mod.get_activation_tables

    def one_set_tables(arch):
        t = dict(orig_tables(arch))
        return {name: (funcs if name == "natural_log_exp_and_others" else frozenset())
                for name, funcs in t.items()}

    bacc_mod.get_activation_tables = one_set_tables
    try:
        nc.compile()
    finally:
        bacc_mod.get_activation_tables = orig_tables
    return nc


def _get_nc(qk_mode, repeats=1):
    key = (qk_mode, repeats)
    if key not in _CACHE:
        _CACHE[key] = _build(qk_mode, repeats)
    return _CACHE[key]


def kernel(x, gn_weight, gn_bias, wq, bq, wk, bk, wv, bv, wo, bo):
    from concourse.bass_utils import run_bass_kernel_spmd

    nc = _get_nc(QK_MODE)

    x = np.asarray(x, dtype=np.float32)
    f32 = lambda a: np.ascontiguousarray(np.asarray(a, dtype=np.float32))

    wq64 = np.asarray(wq, dtype=np.float64)
    wk64 = np.asarray(wk, dtype=np.float64)
    wv64 = np.asarray(wv, dtype=np.float64)
    wo64 = np.asarray(wo, dtype=np.float64)
    wu64 = wo64 @ wv64                      # fused v + out conv
    bu = wo64 @ np.asarray(bv, np.float64) + np.asarray(bo, np.float64)

    wqT = f32(wq64.T)
    wkT = f32(wk64.T)
    wuT = f32(wu64.T)
    biases = f32(np.stack([np.asarray(bq, np.float64), bu], axis=1))  # [C, 2]
    gn_ab = f32(np.stack([gn_weight, gn_bias], axis=1))               # [C, 2]

    gsel = np.zeros((C, NUM_GROUPS), dtype=np.float32)
    gsel[np.arange(C), np.arange(C) // GSIZE] = 1.0 / GSIZE
    esel = np.zeros((NUM_GROUPS, C), dtype=np.float32)
    esel[np.arange(C) // GSIZE, np.arange(C)] = 1.0

    in_maps = []
    for core in range(8):
        b, half = core // 2, core % 2
        xb = x[b].reshape(C, N)
        xqb = f32(xb[:, half * NQ:(half + 1) * NQ])
        xob = f32(xb[:, (1 - half) * NQ:(2 - half) * NQ])
        in_maps.append({
            "xq": xqb, "xo": xob, "xqT": f32(xqb.T),
            "wqT": wqT, "wkT": wkT, "wuT": wuT,
            "biases": biases, "gn_ab": gn_ab, "gsel": gsel, "esel": esel,
        })

    import os
    trace = bool(os.environ.get("BASS_TRACE"))
    res = run_bass_kernel_spmd(nc, in_maps, core_ids=list(range(8)),
                               trace=trace)
    global _LAST
    _LAST = res

    out = np.empty((B, C, N), dtype=np.float32)
    for core in range(8):
        b, half = core // 2, core % 2
        out[b, :, half * NQ:(half + 1) * NQ] = res.results[core]["out"].T
    return out.reshape(B, C, H, W)
